# revision 24
# baseline (speedup 1.0000x reference)
"""Criss-cross attention (CC module) Trainium2 Bass kernel, v5 (c-major).

v5 vs v4: pass-2 V comes from SBUF gathers of vth2/vt1 (no x?pw recompute);
colsum Z computed as [128,32] per chunk layout via N=1 matmuls, reciprocal
on [128,64] (full DVE lanes), PE-transposed to [32,128] rows, broadcast back
via all-ones matmul; combines are single [128,512] DVE ops using the merged
(cb, wp, h) access pattern on partT.

Index maps (per chunk j of 128 spatial positions):
  h-major chunk j: col = kc*128 + hp*64 + em, em=(w%2)*32+w//2, h=2j+hp
  att1 chunk j2 quadrant (wp,wp): [h' plain, h plain] for w=2*j2+wp
  att2 chunk j  quadrant (hp,hp): [W' em, w em] for h=2j+hp
  vtw row wq*64+h, col j2*512+c = v[c, h, 2*j2+wq]
  partT col = cb*4096 + j*128 + hp*64 + em ; y2t col s'' = j2*128 + wp*64 + h
"""

import numpy as np
import ml_dtypes

import concourse.bass as bass
import concourse.mybir as mybir
import concourse.tile as tile
from concourse import bacc
from concourse.bass_utils import run_bass_kernel_spmd
from concourse.masks import make_identity

BF16 = mybir.dt.bfloat16
F32 = mybir.dt.float32

B, C, H, W = 8, 512, 64, 64
CQ = 64
S = H * W  # 4096
NCH = S // 128  # 32 spatial chunks of 128
KC = C // 128  # 4 contraction chunks

_CACHED = {}


def build_nc():
    nc = bacc.Bacc("TRN2", target_bir_lowering=False, debug=False)

    x2p = nc.dram_tensor("x2p", [NCH, 128, 512], BF16, kind="ExternalInput")
    x1p = nc.dram_tensor("x1p", [NCH, 128, 512], BF16, kind="ExternalInput")
    x2pw = nc.dram_tensor("x2pw", [NCH, 128, 512], BF16, kind="ExternalInput")
    x1pw = nc.dram_tensor("x1pw", [NCH, 128, 512], BF16, kind="ExternalInput")
    qkw = nc.dram_tensor("qkw", [KC, 128, 128], BF16, kind="ExternalInput")
    qkb2 = nc.dram_tensor("qkb2", [64, 2], BF16, kind="ExternalInput")
    vwtb = nc.dram_tensor("vwtb", [KC, 128, C], BF16, kind="ExternalInput")

    rr_d = nc.dram_tensor("rr_d", [32, 128], BF16)
    y2t = nc.dram_tensor("y2t", [C, S], BF16, kind="ExternalOutput")
    y1t = nc.dram_tensor("y1t", [C, S], BF16, kind="ExternalOutput")

    x2p_v = x2p.rearrange("j p c -> p j c")
    x1p_v = x1p.rearrange("j p c -> p j c")
    x2pw_v = x2pw.rearrange("j p c -> p j c")
    x1pw_v = x1pw.rearrange("j p c -> p j c")
    y2t_v = y2t.rearrange("(cb p) (gi sg) -> gi p cb sg", p=128, sg=512)
    y1t_v = y1t.rearrange("(cb p) (gi sg) -> gi p cb sg", p=128, sg=512)

    with tile.TileContext(nc) as tc:
        with (
            tc.tile_pool(name="persist", bufs=1) as pp,
            tc.tile_pool(name="psA", bufs=6, space="PSUM") as psA,
            tc.tile_pool(name="psB", bufs=2, space="PSUM") as psB,
        ):
            # ---- persistent tiles ----
            qkw_t = [pp.tile([128, 128], BF16, tag=f"qkw_{i}", name=f"qkw_{i}") for i in range(KC)]
            vwtb_t = [pp.tile([128, C], BF16, tag=f"vwtb_{i}", name=f"vwtb_{i}") for i in range(KC)]
            qkb_t = pp.tile([64, 2], BF16, tag="qkb", name="qkb")
            ones_col = pp.tile([128, 1], BF16, tag="ones_col", name="ones_col")
            ident = pp.tile([128, 128], BF16, tag="ident", name="ident")
            att1 = pp.tile([128, S], BF16, tag="att1", name="att1")
            partT2 = pp.tile([128, 4 * S], BF16, tag="partT2", name="partT2")
            partT1 = pp.tile([128, 4 * S], BF16, tag="partT1", name="partT1")
            zr_sb = pp.tile([128, 32], BF16, tag="zr_sb", name="zr_sb")
            t32a = pp.tile([32, 128], BF16, tag="t32a", name="t32a")

            nc.gpsimd.memset(ones_col[:], 1.0)
            nc.vector.memset(att1[:], 0.0)
            make_identity(nc, ident[:])

            nc.scalar.dma_start(qkb_t[:], qkb2[:])
            for i in range(KC):
                nc.sync.dma_start(qkw_t[i][:], qkw[i, :, :])
                nc.gpsimd.dma_start(vwtb_t[i][:], vwtb[i, :, :])

            pool2 = tc.alloc_tile_pool(name="pool2", bufs=1)
            vth2 = pool2.tile([128, NCH * 512], BF16, tag="vth2", name="vth2")
            att2 = pool2.tile([128, S], BF16, tag="att2", name="att2")
            nc.vector.memset(att2[:], 0.0)
            qkpool = tc.alloc_tile_pool(name="qkpool", bufs=1)
            q_sb = qkpool.tile([64, S], BF16, tag="q_sb", name="q_sb")
            k_sb = qkpool.tile([64, S], BF16, tag="k_sb", name="k_sb")
            ringA = tc.alloc_tile_pool(name="ringA", bufs=2)

            # ---- proj pass over x2: Q/K, V2, E_W (1 chunk behind), gathers ----
            psE2 = {}

            def emit_ew(jd):
                gd = jd // 4
                if jd % 4 == 0:
                    psE2[gd] = psA.tile([128, 512], F32, tag="ps", name="psE2")
                for hp in range(2):
                    sl = slice(128 * jd + 64 * hp, 128 * jd + 64 * hp + 64)
                    nc.tensor.matmul(
                        psE2[gd][64 * hp: 64 * hp + 64,
                                 128 * (jd % 4) + 64 * hp: 128 * (jd % 4) + 64 * hp + 64],
                        lhsT=k_sb[:, sl], rhs=q_sb[:, sl],
                        start=True, stop=True, skip_group_check=True,
                        tile_position=(0, 64 * hp),
                    )
                if jd % 4 == 3:
                    att2_g = att2[:].rearrange("p (j hq) -> p j hq", hq=128)
                    psE2_g = psE2[gd][:].rearrange("p (jl hq) -> p jl hq", hq=128)
                    for hp in range(2):
                        nc.scalar.activation(
                            out=att2_g[64 * hp: 64 * hp + 64, 4 * gd: 4 * gd + 4,
                                       64 * hp: 64 * hp + 64],
                            in_=psE2_g[64 * hp: 64 * hp + 64, :, 64 * hp: 64 * hp + 64],
                            func=mybir.ActivationFunctionType.Exp,
                        )

            for j in range(NCH):
                if j % 4 == 0:
                    x2c = ringA.tile([128, 4 * 512], BF16, tag="x2c", bufs=2, name="x2c")
                    if j == 0:
                        nc.scalar.dma_start(x2c[:, 0:512], x2p_v[:, 0:1, :])
                        nc.scalar.dma_start(x2c[:, 512:2048], x2p_v[:, 1:4, :])
                    else:
                        nc.sync.dma_start(x2c[:], x2p_v[:, j: j + 4, :])
                xc = x2c[:, 512 * (j % 4): 512 * (j % 4 + 1)]
                psQK = psB.tile([64, 256], F32, tag="psqk", name="psqk")
                for kc in range(KC):
                    nc.tensor.matmul(
                        psQK[:, 0:128],
                        lhsT=qkw_t[kc][:, 0:64],
                        rhs=xc[:, 128 * kc: 128 * (kc + 1)],
                        start=(kc == 0), stop=(kc == KC - 1),
                    )
                for kc in range(KC):
                    nc.tensor.matmul(
                        psQK[:, 128:256],
                        lhsT=qkw_t[kc][:, 64:128],
                        rhs=xc[:, 128 * kc: 128 * (kc + 1)],
                        start=(kc == 0), stop=(kc == KC - 1),
                    )
                # V2 projection (PE keeps running; E_W of previous chunk next)
                psV = psA.tile([128, 512], F32, tag="ps", name="psV2")
                for kc in range(KC):
                    nc.tensor.matmul(
                        psV[:],
                        lhsT=xc[:, 128 * kc: 128 * (kc + 1)],
                        rhs=vwtb_t[kc][:],
                        start=(kc == 0), stop=(kc == KC - 1),
                    )
                if j >= 1:
                    emit_ew(j - 1)
                nc.scalar.activation(
                    out=q_sb[:, 128 * j: 128 * (j + 1)], in_=psQK[:, 0:128],
                    func=mybir.ActivationFunctionType.Identity, bias=qkb_t[:, 0:1],
                )
                nc.scalar.activation(
                    out=k_sb[:, 128 * j: 128 * (j + 1)], in_=psQK[:, 128:256],
                    func=mybir.ActivationFunctionType.Identity, bias=qkb_t[:, 1:2],
                )
                nc.vector.tensor_copy(vth2[:, 512 * j: 512 * (j + 1)], psV[:])
            emit_ew(NCH - 1)

            # ---- E_H (64 mms) ----
            k_col = k_sb[:].rearrange("p (j hp em) -> p em (j hp)", hp=2, em=64)
            q_col = q_sb[:].rearrange("p (j hp em) -> p em (j hp)", hp=2, em=64)
            att1_g = att1[:].rearrange("p (j hq) -> p j hq", hq=128)
            for g8 in range(8):
                psE1 = psA.tile([128, 512], F32, tag="ps", name="psE1")
                psE1_g = psE1[:].rearrange("p (jl hq) -> p jl hq", hq=128)
                for wl in range(8):
                    w = 8 * g8 + wl
                    wp, j2l = w % 2, (w // 2) % 4
                    em = (w % 2) * 32 + w // 2
                    nc.tensor.matmul(
                        psE1[64 * wp: 64 * wp + 64,
                             128 * j2l + 64 * wp: 128 * j2l + 64 * wp + 64],
                        lhsT=k_col[:, em, :], rhs=q_col[:, em, :],
                        start=True, stop=True, skip_group_check=True,
                        tile_position=(0, 64 * wp),
                    )
                for wp in range(2):
                    nc.scalar.activation(
                        out=att1_g[64 * wp: 64 * wp + 64, 4 * g8: 4 * g8 + 4,
                                   64 * wp: 64 * wp + 64],
                        in_=psE1_g[64 * wp: 64 * wp + 64, :, 64 * wp: 64 * wp + 64],
                        func=mybir.ActivationFunctionType.Exp,
                    )

            ringA.release()
            qkpool.release()
            ringQ = tc.alloc_tile_pool(name="ringQ", bufs=2)
            ringP = tc.alloc_tile_pool(name="ringP", bufs=2)

            # ---- pass-1 V1 pipeline (prologue emitted before normalize) ----
            PD = 20
            x1c_t = {}
            vt1_t = {}

            def emit_v1(jd):
                if jd % 4 == 0:
                    x1c_t[jd // 4] = ringP.tile(
                        [128, 4 * 512], BF16, tag="x1c", bufs=2, name="x1c"
                    )
                    nc.sync.dma_start(x1c_t[jd // 4][:], x1p_v[:, jd: jd + 4, :])
                xc = x1c_t[jd // 4][:, 512 * (jd % 4): 512 * (jd % 4 + 1)]
                psV = psA.tile([128, 512], F32, tag="ps", name="psV1")
                for kc in range(KC):
                    nc.tensor.matmul(
                        psV[:], lhsT=xc[:, 128 * kc: 128 * (kc + 1)],
                        rhs=vwtb_t[kc][:],
                        start=(kc == 0), stop=(kc == KC - 1),
                    )
                vt1 = ringP.tile([128, 512], BF16, tag="vt1", bufs=PD + 2, name="vt1")
                vt1_t[jd] = vt1
                nc.scalar.activation(out=vt1[:], in_=psV[:],
                                     func=mybir.ActivationFunctionType.Copy)

            # ---- colsum Z (att1 layout) -> recip -> transpose -> broadcast ----
            att2_zv = att2[:].rearrange(
                "p (j hp wp j2) -> p j2 wp (j hp)", hp=2, wp=2, j2=32
            )
            psZT = psA.tile([128, 512], F32, tag="ps", name="psZT")
            for j2 in range(NCH):
                nc.tensor.matmul(
                    psZT[:, j2: j2 + 1], lhsT=att1[:, 128 * j2: 128 * (j2 + 1)],
                    rhs=ones_col[:], start=True, stop=False, skip_group_check=True,
                )
                for wp in range(2):
                    nc.tensor.matmul(
                        psZT[64 * wp: 64 * wp + 64, j2: j2 + 1],
                        lhsT=att2_zv[:, j2, wp, :],
                        rhs=ones_col[:], start=False, stop=(wp == 1),
                        skip_group_check=True,
                    )
            with nc.allow_low_precision(reason="softmax recip in bf16"):
                nc.vector.reciprocal(zr_sb[:], psZT[:, 0:32])
            for jd in range(4):
                emit_v1(jd)
            psT1 = psA.tile([128, 1024], BF16, tag="ps", name="psT1")
            nc.tensor.transpose(psT1[0:32, 0:128], zr_sb[:], ident[:])
            nc.vector.tensor_copy(t32a[:], psT1[0:32, 0:128])
            rrow = partT1[0:1, 0:4096]
            nc.sync.dma_start(rr_d[:], t32a[:])
            nc.sync.dma_start(rrow, rr_d.rearrange("a b -> (a b)"))
            for jd in range(4, PD):
                emit_v1(jd)

            # normalize: one big partition broadcast, then per-chunk muls
            rfull = partT2[:, 0:4096]
            nc.gpsimd.partition_broadcast(rfull, rrow)
            rf_v = rfull.rearrange(
                "p (j2 wp j hp) -> p j hp wp j2", j2=32, wp=2, hp=2
            )
            for j in range(NCH):
                eng = nc.vector if j % 2 == 0 else nc.gpsimd
                eng.tensor_mul(
                    att2[:, 128 * j: 128 * (j + 1)],
                    att2[:, 128 * j: 128 * (j + 1)],
                    rf_v[:, j],
                )
            for n in range(8):
                cols = slice(512 * n, 512 * (n + 1))
                nc.vector.tensor_mul(att1[:, cols], att1[:, cols], rfull[:, cols])

            pT2_v = partT2[:].rearrange("p (cb s) -> p cb s", cb=4)
            pT1_v = partT1[:].rearrange("p (cb s) -> p cb s", cb=4)

            # ---- pass 1 main: att_W partials (V1 pipeline PD ahead) ----
            wc_t = {}
            for j in range(NCH):
                if j + PD < NCH:
                    emit_v1(j + PD)
                if j == 26:
                    wc_t["x2wc0"] = ringQ.tile([128, 4 * 512], BF16, tag="x2wc",
                                               bufs=2, name="x2wc")
                    nc.sync.dma_start(wc_t["x2wc0"][:], x2pw_v[:, 0: 4, :])
                if j == 28:
                    wc_t["x1wc0"] = ringQ.tile([128, 4 * 512], BF16, tag="x1wc",
                                               bufs=2, name="x1wc")
                    nc.sync.dma_start(wc_t["x1wc0"][:], x1pw_v[:, 0: 4, :])
                att2c = att2[:, 128 * j: 128 * (j + 1)]
                psO2 = psA.tile([128, 512], F32, tag="ps", name="psO2")
                for cb in range(4):
                    nc.tensor.matmul(
                        psO2[:, 128 * cb: 128 * (cb + 1)],
                        lhsT=vth2[:, 512 * j + 128 * cb: 512 * j + 128 * (cb + 1)],
                        rhs=att2c, start=True, stop=True, skip_group_check=True,
                    )
                nc.vector.tensor_copy(pT2_v[:, :, 128 * j: 128 * (j + 1)], psO2[:])
                psO1 = psA.tile([128, 512], F32, tag="ps", name="psO1")
                for cb in range(4):
                    nc.tensor.matmul(
                        psO1[:, 128 * cb: 128 * (cb + 1)],
                        lhsT=vt1_t[j][:, 128 * cb: 128 * (cb + 1)],
                        rhs=att2c, start=True, stop=True, skip_group_check=True,
                    )
                nc.scalar.activation(out=pT1_v[:, :, 128 * j: 128 * (j + 1)], in_=psO1[:],
                                     func=mybir.ActivationFunctionType.Copy)

            ringP.release()

            # ---- pass 2: att_H + combine; DVE does y2, ACT+gpsimd do y1 ----
            pT2_c = partT2[:].rearrange(
                "p (cb m wp j2) -> p j2 cb wp m", cb=4, m=64, wp=2, j2=32
            )
            pT1_c = partT1[:].rearrange(
                "p (cb m wp j2) -> p j2 cb wp m", cb=4, m=64, wp=2, j2=32
            )
            for j2 in range(NCH):
                gi, jj = j2 // 4, j2 % 4
                if jj == 0:
                    if j2 == 0:
                        x2wc = wc_t["x2wc0"]
                        x1wc = wc_t["x1wc0"]
                    else:
                        x2wc = ringQ.tile([128, 4 * 512], BF16, tag="x2wc", bufs=2, name="x2wc")
                        nc.sync.dma_start(x2wc[:], x2pw_v[:, j2: j2 + 4, :])
                        x1wc = ringQ.tile([128, 4 * 512], BF16, tag="x1wc", bufs=2, name="x1wc")
                        nc.sync.dma_start(x1wc[:], x1pw_v[:, j2: j2 + 4, :])
                    ys2 = ringQ.tile([128, 4 * 512], BF16, tag="ys2", bufs=2, name="ys2")
                    ys1 = ringQ.tile([128, 4 * 512], BF16, tag="ys1", bufs=2, name="ys1")
                att1c = att1[:, 128 * j2: 128 * (j2 + 1)]
                psVw2 = psA.tile([128, 512], F32, tag="ps", name="psVw2")
                for kc in range(KC):
                    nc.tensor.matmul(
                        psVw2[:], lhsT=x2wc[:, 512 * jj + 128 * kc: 512 * jj + 128 * (kc + 1)],
                        rhs=vwtb_t[kc][:],
                        start=(kc == 0), stop=(kc == KC - 1),
                    )
                vt2w = ringQ.tile([128, 512], BF16, tag="vt2w", bufs=3, name="vt2w")
                nc.scalar.activation(out=vt2w[:], in_=psVw2[:],
                                     func=mybir.ActivationFunctionType.Copy)
                psVw1 = psA.tile([128, 512], F32, tag="ps", name="psVw1")
                for kc in range(KC):
                    nc.tensor.matmul(
                        psVw1[:], lhsT=x1wc[:, 512 * jj + 128 * kc: 512 * jj + 128 * (kc + 1)],
                        rhs=vwtb_t[kc][:],
                        start=(kc == 0), stop=(kc == KC - 1),
                    )
                vt1w = ringQ.tile([128, 512], BF16, tag="vt1w", bufs=3, name="vt1w")
                nc.vector.tensor_copy(vt1w[:], psVw1[:])
                psF2 = psA.tile([128, 512], F32, tag="ps", name="psF2")
                for cb in range(4):
                    nc.tensor.matmul(
                        psF2[:, 128 * cb: 128 * (cb + 1)],
                        lhsT=vt2w[:, 128 * cb: 128 * (cb + 1)],
                        rhs=att1c, start=True, stop=True, skip_group_check=True,
                    )
                psF1 = psA.tile([128, 512], F32, tag="ps", name="psF1")
                for cb in range(4):
                    nc.tensor.matmul(
                        psF1[:, 128 * cb: 128 * (cb + 1)],
                        lhsT=vt1w[:, 128 * cb: 128 * (cb + 1)],
                        rhs=att1c, start=True, stop=True, skip_group_check=True,
                    )
                ys2_v = ys2[:].rearrange(
                    "p (cb jl wp m) -> p jl cb wp m", cb=4, jl=4, wp=2, m=64
                )
                ys1_v = ys1[:].rearrange(
                    "p (cb jl wp m) -> p jl cb wp m", cb=4, jl=4, wp=2, m=64
                )
                psF2_v = psF2[:].rearrange("p (cb wp m) -> p cb wp m", cb=4, wp=2, m=64)
                nc.vector.tensor_add(ys2_v[:, jj], psF2_v, pT2_c[:, j2])
                f1tmp = ringQ.tile([128, 512], BF16, tag="f1tmp", bufs=3, name="f1tmp")
                nc.scalar.activation(out=f1tmp[:], in_=psF1[:],
                                     func=mybir.ActivationFunctionType.Copy)
                f1_v = f1tmp[:].rearrange("p (cb wp m) -> p cb wp m", cb=4, wp=2, m=64)
                nc.gpsimd.tensor_add(ys1_v[:, jj], f1_v, pT1_c[:, j2])
                if j2 < 24 and jj == 3:
                    nc.gpsimd.dma_start(y2t_v[gi], ys2[:])
                    nc.sync.dma_start(y1t_v[gi], ys1[:])
                elif j2 >= 24 and j2 % 2 == 1:
                    half = (jj - 1) // 2
                    y2h = y2t_v[gi].rearrange("p cb (hf sg) -> p hf cb sg", hf=2)
                    y1h = y1t_v[gi].rearrange("p cb (hf sg) -> p hf cb sg", hf=2)
                    ys2h = ys2[:].rearrange("p (cb hf sg) -> p hf cb sg", cb=4, hf=2)
                    ys1h = ys1[:].rearrange("p (cb hf sg) -> p hf cb sg", cb=4, hf=2)
                    nc.gpsimd.dma_start(y2h[:, half], ys2h[:, half])
                    nc.sync.dma_start(y1h[:, half], ys1h[:, half])

            ringQ.release()
            pool2.release()

    nc.compile()
    return nc


def make_in_maps(x2, x1, q_w, q_b, k_w, k_b, v_w, v_b, gamma):
    x2 = np.asarray(x2, dtype=np.float32)
    x1 = np.asarray(x1, dtype=np.float32)
    g = float(np.asarray(gamma).reshape(-1)[0])
    bf16 = ml_dtypes.bfloat16
    qkw = (
        np.concatenate([np.asarray(q_w).T, np.asarray(k_w).T], axis=1)
        .reshape(KC, 128, 128).astype(bf16)
    )
    qkb2 = np.stack([np.asarray(q_b), np.asarray(k_b)], axis=1).astype(bf16)
    vwtb = (g * np.asarray(v_w)).T.reshape(KC, 128, C).astype(bf16)

    def pack_p(xfl):
        t = xfl.reshape(KC, 128, NCH, 2, 32, 2)  # kc ch j hp u wpar
        return np.ascontiguousarray(
            t.transpose(2, 1, 0, 3, 5, 4).reshape(NCH, 128, KC * 128).astype(bf16)
        )

    def pack_pw(xfl):
        t = xfl.reshape(KC, 128, 64, 32, 2)  # kc ch h j2 wq
        return np.ascontiguousarray(
            t.transpose(3, 1, 0, 4, 2).reshape(NCH, 128, KC * 128).astype(bf16)
        )

    in_maps = []
    for b in range(B):
        x2fl = x2[b].reshape(C, S)
        x1fl = x1[b].reshape(C, S)
        in_maps.append(
            {
                "x2p": pack_p(x2fl),
                "x1p": pack_p(x1fl),
                "x2pw": pack_pw(x2fl),
                "x1pw": pack_pw(x1fl),
                "qkw": qkw,
                "qkb2": qkb2,
                "vwtb": vwtb,
            }
        )
    return in_maps, g


def assemble_outputs(res, x2, x1, v_b, g):
    y2 = np.empty((B, C, H, W), np.float32)
    y1 = np.empty((B, C, H, W), np.float32)
    gvb = (g * np.asarray(v_b, dtype=np.float32))[None, :, None, None]
    for b in range(B):
        y2[b] = unpermute(np.asarray(res[b]["y2t"]))
        y1[b] = unpermute(np.asarray(res[b]["y1t"]))
    y2 += gvb
    y2 += np.asarray(x2, dtype=np.float32)
    y1 += gvb
    y1 += np.asarray(x1, dtype=np.float32)
    return y2, y1


def unpermute(yt):
    # yt [C, s''=j2*128+wp*64+h] -> y[c, h, w=2*j2+wp]
    return np.ascontiguousarray(
        yt.astype(np.float32).reshape(C, 32, 2, 64).transpose(0, 3, 1, 2).reshape(C, H, W)
    )


def kernel(x2, x1, q_w, q_b, k_w, k_b, v_w, v_b, gamma):
    in_maps, g = make_in_maps(x2, x1, q_w, q_b, k_w, k_b, v_w, v_b, gamma)
    if "nc" not in _CACHED:
        _CACHED["nc"] = build_nc()
    nc = _CACHED["nc"]
    res = run_bass_kernel_spmd(nc, in_maps, list(range(B))).results
    return assemble_outputs(res, x2, x1, v_b, g)


# revision 25
# speedup vs baseline: 1.1641x; 1.1641x over previous
"""Criss-cross attention (CC module) Trainium2 Bass kernel, v5 (c-major).

v5 vs v4: pass-2 V comes from SBUF gathers of vth2/vt1 (no x?pw recompute);
colsum Z computed as [128,32] per chunk layout via N=1 matmuls, reciprocal
on [128,64] (full DVE lanes), PE-transposed to [32,128] rows, broadcast back
via all-ones matmul; combines are single [128,512] DVE ops using the merged
(cb, wp, h) access pattern on partT.

Index maps (per chunk j of 128 spatial positions):
  h-major chunk j: col = kc*128 + hp*64 + em, em=(w%2)*32+w//2, h=2j+hp
  att1 chunk j2 quadrant (wp,wp): [h' plain, h plain] for w=2*j2+wp
  att2 chunk j  quadrant (hp,hp): [W' em, w em] for h=2j+hp
  vtw row wq*64+h, col j2*512+c = v[c, h, 2*j2+wq]
  partT col = cb*4096 + j*128 + hp*64 + em ; y2t col s'' = j2*128 + wp*64 + h
"""

import numpy as np
import ml_dtypes

import concourse.bass as bass
import concourse.mybir as mybir
import concourse.tile as tile
from concourse import bacc
from concourse.bass_utils import run_bass_kernel_spmd
from concourse.masks import make_identity

BF16 = mybir.dt.bfloat16
F32 = mybir.dt.float32

B, C, H, W = 8, 512, 64, 64
CQ = 64
S = H * W  # 4096
NCH = S // 128  # 32 spatial chunks of 128
KC = C // 128  # 4 contraction chunks

_CACHED = {}


def build_nc():
    nc = bacc.Bacc("TRN2", target_bir_lowering=False, debug=False)

    x2p = nc.dram_tensor("x2p", [NCH, 128, 512], BF16, kind="ExternalInput")
    x1p = nc.dram_tensor("x1p", [NCH, 128, 512], BF16, kind="ExternalInput")
    x2pw = nc.dram_tensor("x2pw", [NCH, 128, 512], BF16, kind="ExternalInput")
    x1pw = nc.dram_tensor("x1pw", [NCH, 128, 512], BF16, kind="ExternalInput")
    qkw = nc.dram_tensor("qkw", [KC, 128, 128], BF16, kind="ExternalInput")
    qkb2 = nc.dram_tensor("qkb2", [64, 2], BF16, kind="ExternalInput")
    vwtb = nc.dram_tensor("vwtb", [KC, 128, C], BF16, kind="ExternalInput")

    rr_d = nc.dram_tensor("rr_d", [32, 128], BF16)
    y2t = nc.dram_tensor("y2t", [C, S], BF16, kind="ExternalOutput")
    y1t = nc.dram_tensor("y1t", [C, S], BF16, kind="ExternalOutput")

    x2p_v = x2p.rearrange("j p c -> p j c")
    x1p_v = x1p.rearrange("j p c -> p j c")
    x2pw_v = x2pw.rearrange("j p c -> p j c")
    x1pw_v = x1pw.rearrange("j p c -> p j c")
    y2t_v = y2t.rearrange("(cb p) (gi sg) -> gi p cb sg", p=128, sg=512)
    y1t_v = y1t.rearrange("(cb p) (gi sg) -> gi p cb sg", p=128, sg=512)

    with tile.TileContext(nc) as tc:
        with (
            tc.tile_pool(name="persist", bufs=1) as pp,
            tc.tile_pool(name="psA", bufs=6, space="PSUM") as psA,
            tc.tile_pool(name="psB", bufs=2, space="PSUM") as psB,
        ):
            # ---- persistent tiles ----
            qkw_t = [pp.tile([128, 128], BF16, tag=f"qkw_{i}", name=f"qkw_{i}") for i in range(KC)]
            vwtb_t = [pp.tile([128, C], BF16, tag=f"vwtb_{i}", name=f"vwtb_{i}") for i in range(KC)]
            qkb_t = pp.tile([64, 2], BF16, tag="qkb", name="qkb")
            ones_col = pp.tile([128, 1], BF16, tag="ones_col", name="ones_col")
            ident = pp.tile([128, 128], BF16, tag="ident", name="ident")
            att1 = pp.tile([128, S], BF16, tag="att1", name="att1")
            partT2 = pp.tile([128, 4 * S], BF16, tag="partT2", name="partT2")
            partT1 = pp.tile([128, 4 * S], BF16, tag="partT1", name="partT1")
            zr_sb = pp.tile([128, 32], BF16, tag="zr_sb", name="zr_sb")
            t32a = pp.tile([32, 128], BF16, tag="t32a", name="t32a")

            nc.gpsimd.memset(ones_col[:], 1.0)
            nc.vector.memset(att1[:], 0.0)
            make_identity(nc, ident[:])

            nc.scalar.dma_start(qkb_t[:], qkb2[:])
            for i in range(KC):
                nc.sync.dma_start(qkw_t[i][:], qkw[i, :, :])
                nc.gpsimd.dma_start(vwtb_t[i][:], vwtb[i, :, :])

            pool2 = tc.alloc_tile_pool(name="pool2", bufs=1)
            vth2 = pool2.tile([128, NCH * 512], BF16, tag="vth2", name="vth2")
            att2 = pool2.tile([128, S], BF16, tag="att2", name="att2")
            nc.vector.memset(att2[:], 0.0)
            qkpool = tc.alloc_tile_pool(name="qkpool", bufs=1)
            q_sb = qkpool.tile([64, S], BF16, tag="q_sb", name="q_sb")
            k_sb = qkpool.tile([64, S], BF16, tag="k_sb", name="k_sb")
            ringA = tc.alloc_tile_pool(name="ringA", bufs=2)

            # ---- proj pass over x2: Q/K, V2, E_W (1 chunk behind), gathers ----
            psE2 = {}

            def emit_ew(jd):
                gd = jd // 4
                if jd % 4 == 0:
                    psE2[gd] = psA.tile([128, 512], F32, tag="ps", name="psE2")
                for hp in range(2):
                    sl = slice(128 * jd + 64 * hp, 128 * jd + 64 * hp + 64)
                    nc.tensor.matmul(
                        psE2[gd][64 * hp: 64 * hp + 64,
                                 128 * (jd % 4) + 64 * hp: 128 * (jd % 4) + 64 * hp + 64],
                        lhsT=k_sb[:, sl], rhs=q_sb[:, sl],
                        start=True, stop=True, skip_group_check=True,
                        tile_position=(0, 64 * hp),
                    )
                if jd % 4 == 3:
                    att2_g = att2[:].rearrange("p (j hq) -> p j hq", hq=128)
                    psE2_g = psE2[gd][:].rearrange("p (jl hq) -> p jl hq", hq=128)
                    for hp in range(2):
                        nc.scalar.activation(
                            out=att2_g[64 * hp: 64 * hp + 64, 4 * gd: 4 * gd + 4,
                                       64 * hp: 64 * hp + 64],
                            in_=psE2_g[64 * hp: 64 * hp + 64, :, 64 * hp: 64 * hp + 64],
                            func=mybir.ActivationFunctionType.Exp,
                        )

            for j in range(NCH):
                if j % 4 == 0:
                    x2c = ringA.tile([128, 4 * 512], BF16, tag="x2c", bufs=2, name="x2c")
                    if j == 0:
                        nc.scalar.dma_start(x2c[:, 0:512], x2p_v[:, 0:1, :])
                        nc.scalar.dma_start(x2c[:, 512:2048], x2p_v[:, 1:4, :])
                    else:
                        nc.sync.dma_start(x2c[:], x2p_v[:, j: j + 4, :])
                xc = x2c[:, 512 * (j % 4): 512 * (j % 4 + 1)]
                psQK = psB.tile([64, 256], F32, tag="psqk", name="psqk")
                for kc in range(KC):
                    nc.tensor.matmul(
                        psQK[:, 0:128],
                        lhsT=qkw_t[kc][:, 0:64],
                        rhs=xc[:, 128 * kc: 128 * (kc + 1)],
                        start=(kc == 0), stop=(kc == KC - 1),
                    )
                for kc in range(KC):
                    nc.tensor.matmul(
                        psQK[:, 128:256],
                        lhsT=qkw_t[kc][:, 64:128],
                        rhs=xc[:, 128 * kc: 128 * (kc + 1)],
                        start=(kc == 0), stop=(kc == KC - 1),
                    )
                # V2 projection (PE keeps running; E_W of previous chunk next)
                psV = psA.tile([128, 512], F32, tag="ps", name="psV2")
                for kc in range(KC):
                    nc.tensor.matmul(
                        psV[:],
                        lhsT=xc[:, 128 * kc: 128 * (kc + 1)],
                        rhs=vwtb_t[kc][:],
                        start=(kc == 0), stop=(kc == KC - 1),
                    )
                if j >= 1:
                    emit_ew(j - 1)
                nc.scalar.activation(
                    out=q_sb[:, 128 * j: 128 * (j + 1)], in_=psQK[:, 0:128],
                    func=mybir.ActivationFunctionType.Identity, bias=qkb_t[:, 0:1],
                )
                nc.scalar.activation(
                    out=k_sb[:, 128 * j: 128 * (j + 1)], in_=psQK[:, 128:256],
                    func=mybir.ActivationFunctionType.Identity, bias=qkb_t[:, 1:2],
                )
                nc.vector.tensor_copy(vth2[:, 512 * j: 512 * (j + 1)], psV[:])
            emit_ew(NCH - 1)

            # ---- E_H (64 mms) ----
            k_col = k_sb[:].rearrange("p (j hp em) -> p em (j hp)", hp=2, em=64)
            q_col = q_sb[:].rearrange("p (j hp em) -> p em (j hp)", hp=2, em=64)
            att1_g = att1[:].rearrange("p (j hq) -> p j hq", hq=128)
            for g8 in range(8):
                psE1 = psA.tile([128, 512], F32, tag="ps", name="psE1")
                psE1_g = psE1[:].rearrange("p (jl hq) -> p jl hq", hq=128)
                for wl in range(8):
                    w = 8 * g8 + wl
                    wp, j2l = w % 2, (w // 2) % 4
                    em = (w % 2) * 32 + w // 2
                    nc.tensor.matmul(
                        psE1[64 * wp: 64 * wp + 64,
                             128 * j2l + 64 * wp: 128 * j2l + 64 * wp + 64],
                        lhsT=k_col[:, em, :], rhs=q_col[:, em, :],
                        start=True, stop=True, skip_group_check=True,
                        tile_position=(0, 64 * wp),
                    )
                for wp in range(2):
                    nc.scalar.activation(
                        out=att1_g[64 * wp: 64 * wp + 64, 4 * g8: 4 * g8 + 4,
                                   64 * wp: 64 * wp + 64],
                        in_=psE1_g[64 * wp: 64 * wp + 64, :, 64 * wp: 64 * wp + 64],
                        func=mybir.ActivationFunctionType.Exp,
                    )

            ringA.release()
            qkpool.release()
            ringP = tc.alloc_tile_pool(name="ringP", bufs=2)

            # ---- pass-1 V1 pipeline (prologue emitted before normalize) ----
            PD = 20
            x1c_t = {}
            vt1_t = {}

            def emit_v1(jd):
                if jd % 4 == 0:
                    x1c_t[jd // 4] = ringP.tile(
                        [128, 4 * 512], BF16, tag="x1c", bufs=2, name="x1c"
                    )
                    nc.sync.dma_start(x1c_t[jd // 4][:], x1p_v[:, jd: jd + 4, :])
                xc = x1c_t[jd // 4][:, 512 * (jd % 4): 512 * (jd % 4 + 1)]
                psV = psA.tile([128, 512], F32, tag="ps", name="psV1")
                for kc in range(KC):
                    nc.tensor.matmul(
                        psV[:], lhsT=xc[:, 128 * kc: 128 * (kc + 1)],
                        rhs=vwtb_t[kc][:],
                        start=(kc == 0), stop=(kc == KC - 1),
                    )
                vt1 = ringP.tile([128, 512], BF16, tag="vt1", bufs=PD + 2, name="vt1")
                vt1_t[jd] = vt1
                nc.scalar.activation(out=vt1[:], in_=psV[:],
                                     func=mybir.ActivationFunctionType.Copy)

            # ---- colsum Z (att1 layout) -> recip -> transpose -> broadcast ----
            att2_zv = att2[:].rearrange(
                "p (j hp wp j2) -> p j2 wp (j hp)", hp=2, wp=2, j2=32
            )
            psZT = psA.tile([128, 512], F32, tag="ps", name="psZT")
            for j2 in range(NCH):
                nc.tensor.matmul(
                    psZT[:, j2: j2 + 1], lhsT=att1[:, 128 * j2: 128 * (j2 + 1)],
                    rhs=ones_col[:], start=True, stop=False, skip_group_check=True,
                )
                for wp in range(2):
                    nc.tensor.matmul(
                        psZT[64 * wp: 64 * wp + 64, j2: j2 + 1],
                        lhsT=att2_zv[:, j2, wp, :],
                        rhs=ones_col[:], start=False, stop=(wp == 1),
                        skip_group_check=True,
                    )
            with nc.allow_low_precision(reason="softmax recip in bf16"):
                nc.vector.reciprocal(zr_sb[:], psZT[:, 0:32])
            for jd in range(4):
                emit_v1(jd)
            psT1 = psA.tile([128, 1024], BF16, tag="ps", name="psT1")
            nc.tensor.transpose(psT1[0:32, 0:128], zr_sb[:], ident[:])
            nc.vector.tensor_copy(t32a[:], psT1[0:32, 0:128])
            rrow = partT1[0:1, 0:4096]
            nc.sync.dma_start(rr_d[:], t32a[:])
            nc.sync.dma_start(rrow, rr_d.rearrange("a b -> (a b)"))
            for jd in range(4, PD):
                emit_v1(jd)

            # normalize: one big partition broadcast, then per-chunk muls
            rfull = partT2[:, 0:4096]
            nc.gpsimd.partition_broadcast(rfull, rrow)
            rf_v = rfull.rearrange(
                "p (j2 wp j hp) -> p j hp wp j2", j2=32, wp=2, hp=2
            )
            for j in range(NCH):
                eng = nc.vector if j % 2 == 0 else nc.gpsimd
                eng.tensor_mul(
                    att2[:, 128 * j: 128 * (j + 1)],
                    att2[:, 128 * j: 128 * (j + 1)],
                    rf_v[:, j],
                )
            for n in range(8):
                cols = slice(512 * n, 512 * (n + 1))
                nc.vector.tensor_mul(att1[:, cols], att1[:, cols], rfull[:, cols])

            pT2_v = partT2[:].rearrange("p (cb s) -> p cb s", cb=4)
            pT1_v = partT1[:].rearrange("p (cb s) -> p cb s", cb=4)

            # ---- pass 1 main: att_W partials (V1 pipeline PD ahead) ----
            for j in range(NCH):
                if j + PD < NCH:
                    emit_v1(j + PD)
                att2c = att2[:, 128 * j: 128 * (j + 1)]
                psO2 = psA.tile([128, 512], F32, tag="ps", name="psO2")
                for cb in range(4):
                    nc.tensor.matmul(
                        psO2[:, 128 * cb: 128 * (cb + 1)],
                        lhsT=vth2[:, 512 * j + 128 * cb: 512 * j + 128 * (cb + 1)],
                        rhs=att2c, start=True, stop=True, skip_group_check=True,
                    )
                nc.vector.tensor_copy(pT2_v[:, :, 128 * j: 128 * (j + 1)], psO2[:])
                psO1 = psA.tile([128, 512], F32, tag="ps", name="psO1")
                for cb in range(4):
                    nc.tensor.matmul(
                        psO1[:, 128 * cb: 128 * (cb + 1)],
                        lhsT=vt1_t[j][:, 128 * cb: 128 * (cb + 1)],
                        rhs=att2c, start=True, stop=True, skip_group_check=True,
                    )
                nc.scalar.activation(out=pT1_v[:, :, 128 * j: 128 * (j + 1)], in_=psO1[:],
                                     func=mybir.ActivationFunctionType.Copy)

            ringP.release()
            pool2.release()
            ringQ = tc.alloc_tile_pool(name="ringQ", bufs=2)

            # ---- pass 2: att_H + combine; DVE does y2, ACT+gpsimd do y1 ----
            pT2_c = partT2[:].rearrange(
                "p (cb m wp j2) -> p j2 cb wp m", cb=4, m=64, wp=2, j2=32
            )
            pT1_c = partT1[:].rearrange(
                "p (cb m wp j2) -> p j2 cb wp m", cb=4, m=64, wp=2, j2=32
            )
            for j2 in range(NCH):
                gi, jj = j2 // 4, j2 % 4
                if jj == 0:
                    x2wc = ringQ.tile([128, 4 * 512], BF16, tag="x2wc", bufs=2, name="x2wc")
                    nc.sync.dma_start(x2wc[:], x2pw_v[:, j2: j2 + 4, :])
                    x1wc = ringQ.tile([128, 4 * 512], BF16, tag="x1wc", bufs=2, name="x1wc")
                    nc.sync.dma_start(x1wc[:], x1pw_v[:, j2: j2 + 4, :])
                    ys2 = ringQ.tile([128, 4 * 512], BF16, tag="ys2", bufs=2, name="ys2")
                    ys1 = ringQ.tile([128, 4 * 512], BF16, tag="ys1", bufs=2, name="ys1")
                att1c = att1[:, 128 * j2: 128 * (j2 + 1)]
                psVw2 = psA.tile([128, 512], F32, tag="ps", name="psVw2")
                for kc in range(KC):
                    nc.tensor.matmul(
                        psVw2[:], lhsT=x2wc[:, 512 * jj + 128 * kc: 512 * jj + 128 * (kc + 1)],
                        rhs=vwtb_t[kc][:],
                        start=(kc == 0), stop=(kc == KC - 1),
                    )
                vt2w = ringQ.tile([128, 512], BF16, tag="vt2w", bufs=3, name="vt2w")
                nc.scalar.activation(out=vt2w[:], in_=psVw2[:],
                                     func=mybir.ActivationFunctionType.Copy)
                psVw1 = psA.tile([128, 512], F32, tag="ps", name="psVw1")
                for kc in range(KC):
                    nc.tensor.matmul(
                        psVw1[:], lhsT=x1wc[:, 512 * jj + 128 * kc: 512 * jj + 128 * (kc + 1)],
                        rhs=vwtb_t[kc][:],
                        start=(kc == 0), stop=(kc == KC - 1),
                    )
                vt1w = ringQ.tile([128, 512], BF16, tag="vt1w", bufs=3, name="vt1w")
                nc.vector.tensor_copy(vt1w[:], psVw1[:])
                psF2 = psA.tile([128, 512], F32, tag="ps", name="psF2")
                for cb in range(4):
                    nc.tensor.matmul(
                        psF2[:, 128 * cb: 128 * (cb + 1)],
                        lhsT=vt2w[:, 128 * cb: 128 * (cb + 1)],
                        rhs=att1c, start=True, stop=True, skip_group_check=True,
                    )
                psF1 = psA.tile([128, 512], F32, tag="ps", name="psF1")
                for cb in range(4):
                    nc.tensor.matmul(
                        psF1[:, 128 * cb: 128 * (cb + 1)],
                        lhsT=vt1w[:, 128 * cb: 128 * (cb + 1)],
                        rhs=att1c, start=True, stop=True, skip_group_check=True,
                    )
                ys2_v = ys2[:].rearrange(
                    "p (cb jl wp m) -> p jl cb wp m", cb=4, jl=4, wp=2, m=64
                )
                ys1_v = ys1[:].rearrange(
                    "p (cb jl wp m) -> p jl cb wp m", cb=4, jl=4, wp=2, m=64
                )
                psF2_v = psF2[:].rearrange("p (cb wp m) -> p cb wp m", cb=4, wp=2, m=64)
                nc.vector.tensor_add(ys2_v[:, jj], psF2_v, pT2_c[:, j2])
                if j2 >= NCH - 2:
                    psF1_v = psF1[:].rearrange("p (cb wp m) -> p cb wp m", cb=4, wp=2, m=64)
                    nc.vector.tensor_add(ys1_v[:, jj], psF1_v, pT1_c[:, j2])
                else:
                    f1tmp = ringQ.tile([128, 512], BF16, tag="f1tmp", bufs=3, name="f1tmp")
                    nc.scalar.activation(out=f1tmp[:], in_=psF1[:],
                                         func=mybir.ActivationFunctionType.Copy)
                    f1_v = f1tmp[:].rearrange("p (cb wp m) -> p cb wp m", cb=4, wp=2, m=64)
                    nc.gpsimd.tensor_add(ys1_v[:, jj], f1_v, pT1_c[:, j2])
                if j2 < 24 and jj == 3:
                    nc.gpsimd.dma_start(y2t_v[gi], ys2[:])
                    nc.sync.dma_start(y1t_v[gi], ys1[:])
                elif j2 >= 24 and j2 % 2 == 1:
                    half = (jj - 1) // 2
                    y2h = y2t_v[gi].rearrange("p cb (hf sg) -> p hf cb sg", hf=2)
                    y1h = y1t_v[gi].rearrange("p cb (hf sg) -> p hf cb sg", hf=2)
                    ys2h = ys2[:].rearrange("p (cb hf sg) -> p hf cb sg", cb=4, hf=2)
                    ys1h = ys1[:].rearrange("p (cb hf sg) -> p hf cb sg", cb=4, hf=2)
                    nc.gpsimd.dma_start(y2h[:, half], ys2h[:, half])
                    nc.sync.dma_start(y1h[:, half], ys1h[:, half])

            ringQ.release()

    nc.compile()
    return nc


def make_in_maps(x2, x1, q_w, q_b, k_w, k_b, v_w, v_b, gamma):
    x2 = np.asarray(x2, dtype=np.float32)
    x1 = np.asarray(x1, dtype=np.float32)
    g = float(np.asarray(gamma).reshape(-1)[0])
    bf16 = ml_dtypes.bfloat16
    qkw = (
        np.concatenate([np.asarray(q_w).T, np.asarray(k_w).T], axis=1)
        .reshape(KC, 128, 128).astype(bf16)
    )
    qkb2 = np.stack([np.asarray(q_b), np.asarray(k_b)], axis=1).astype(bf16)
    vwtb = (g * np.asarray(v_w)).T.reshape(KC, 128, C).astype(bf16)

    def pack_p(xfl):
        t = xfl.reshape(KC, 128, NCH, 2, 32, 2)  # kc ch j hp u wpar
        return np.ascontiguousarray(
            t.transpose(2, 1, 0, 3, 5, 4).reshape(NCH, 128, KC * 128).astype(bf16)
        )

    def pack_pw(xfl):
        t = xfl.reshape(KC, 128, 64, 32, 2)  # kc ch h j2 wq
        return np.ascontiguousarray(
            t.transpose(3, 1, 0, 4, 2).reshape(NCH, 128, KC * 128).astype(bf16)
        )

    in_maps = []
    for b in range(B):
        x2fl = x2[b].reshape(C, S)
        x1fl = x1[b].reshape(C, S)
        in_maps.append(
            {
                "x2p": pack_p(x2fl),
                "x1p": pack_p(x1fl),
                "x2pw": pack_pw(x2fl),
                "x1pw": pack_pw(x1fl),
                "qkw": qkw,
                "qkb2": qkb2,
                "vwtb": vwtb,
            }
        )
    return in_maps, g


def assemble_outputs(res, x2, x1, v_b, g):
    y2 = np.empty((B, C, H, W), np.float32)
    y1 = np.empty((B, C, H, W), np.float32)
    gvb = (g * np.asarray(v_b, dtype=np.float32))[None, :, None, None]
    for b in range(B):
        y2[b] = unpermute(np.asarray(res[b]["y2t"]))
        y1[b] = unpermute(np.asarray(res[b]["y1t"]))
    y2 += gvb
    y2 += np.asarray(x2, dtype=np.float32)
    y1 += gvb
    y1 += np.asarray(x1, dtype=np.float32)
    return y2, y1


def unpermute(yt):
    # yt [C, s''=j2*128+wp*64+h] -> y[c, h, w=2*j2+wp]
    return np.ascontiguousarray(
        yt.astype(np.float32).reshape(C, 32, 2, 64).transpose(0, 3, 1, 2).reshape(C, H, W)
    )


def kernel(x2, x1, q_w, q_b, k_w, k_b, v_w, v_b, gamma):
    in_maps, g = make_in_maps(x2, x1, q_w, q_b, k_w, k_b, v_w, v_b, gamma)
    if "nc" not in _CACHED:
        _CACHED["nc"] = build_nc()
    nc = _CACHED["nc"]
    res = run_bass_kernel_spmd(nc, in_maps, list(range(B))).results
    return assemble_outputs(res, x2, x1, v_b, g)


# revision 26
# speedup vs baseline: 1.2637x; 1.0856x over previous
"""Criss-cross attention (CC module) Trainium2 Bass kernel, v5 (c-major).

v5 vs v4: pass-2 V comes from SBUF gathers of vth2/vt1 (no x?pw recompute);
colsum Z computed as [128,32] per chunk layout via N=1 matmuls, reciprocal
on [128,64] (full DVE lanes), PE-transposed to [32,128] rows, broadcast back
via all-ones matmul; combines are single [128,512] DVE ops using the merged
(cb, wp, h) access pattern on partT.

Index maps (per chunk j of 128 spatial positions):
  h-major chunk j: col = kc*128 + hp*64 + em, em=(w%2)*32+w//2, h=2j+hp
  att1 chunk j2 quadrant (wp,wp): [h' plain, h plain] for w=2*j2+wp
  att2 chunk j  quadrant (hp,hp): [W' em, w em] for h=2j+hp
  vtw row wq*64+h, col j2*512+c = v[c, h, 2*j2+wq]
  partT col = cb*4096 + j*128 + hp*64 + em ; y2t col s'' = j2*128 + wp*64 + h
"""

import numpy as np
import ml_dtypes

import concourse.bass as bass
import concourse.mybir as mybir
import concourse.tile as tile
from concourse import bacc
from concourse.bass_utils import run_bass_kernel_spmd
from concourse.masks import make_identity

BF16 = mybir.dt.bfloat16
F32 = mybir.dt.float32

B, C, H, W = 8, 512, 64, 64
CQ = 64
S = H * W  # 4096
NCH = S // 128  # 32 spatial chunks of 128
KC = C // 128  # 4 contraction chunks

_CACHED = {}


def build_nc():
    nc = bacc.Bacc("TRN2", target_bir_lowering=False, debug=False)

    x2p = nc.dram_tensor("x2p", [NCH, 128, 512], BF16, kind="ExternalInput")
    x1p = nc.dram_tensor("x1p", [NCH, 128, 512], BF16, kind="ExternalInput")
    x2pw = nc.dram_tensor("x2pw", [NCH, 128, 512], BF16, kind="ExternalInput")
    x1pw = nc.dram_tensor("x1pw", [NCH, 128, 512], BF16, kind="ExternalInput")
    qkw = nc.dram_tensor("qkw", [KC, 128, 128], BF16, kind="ExternalInput")
    qkb2 = nc.dram_tensor("qkb2", [64, 2], BF16, kind="ExternalInput")
    vwtb = nc.dram_tensor("vwtb", [KC, 128, C], BF16, kind="ExternalInput")

    rr_d = nc.dram_tensor("rr_d", [32, 128], BF16)
    y2t = nc.dram_tensor("y2t", [C, S], BF16, kind="ExternalOutput")
    y1t = nc.dram_tensor("y1t", [C, S], BF16, kind="ExternalOutput")

    x2p_v = x2p.rearrange("j p c -> p j c")
    x1p_v = x1p.rearrange("j p c -> p j c")
    x2pw_v = x2pw.rearrange("j p c -> p j c")
    x1pw_v = x1pw.rearrange("j p c -> p j c")
    y2t_v = y2t.rearrange("(cb p) (gi sg) -> gi p cb sg", p=128, sg=512)
    y1t_v = y1t.rearrange("(cb p) (gi sg) -> gi p cb sg", p=128, sg=512)

    with tile.TileContext(nc) as tc:
        with (
            tc.tile_pool(name="persist", bufs=1) as pp,
            tc.tile_pool(name="psA", bufs=6, space="PSUM") as psA,
            tc.tile_pool(name="psB", bufs=2, space="PSUM") as psB,
        ):
            # ---- persistent tiles ----
            qkw_t = [pp.tile([128, 128], BF16, tag=f"qkw_{i}", name=f"qkw_{i}") for i in range(KC)]
            vwtb_t = [pp.tile([128, C], BF16, tag=f"vwtb_{i}", name=f"vwtb_{i}") for i in range(KC)]
            qkb_t = pp.tile([64, 2], BF16, tag="qkb", name="qkb")
            ones_col = pp.tile([128, 1], BF16, tag="ones_col", name="ones_col")
            ident = pp.tile([128, 128], BF16, tag="ident", name="ident")
            att1 = pp.tile([128, S], BF16, tag="att1", name="att1")
            partT2 = pp.tile([128, 4 * S], BF16, tag="partT2", name="partT2")
            partT1 = pp.tile([128, 4 * S], BF16, tag="partT1", name="partT1")
            zr_sb = pp.tile([128, 32], BF16, tag="zr_sb", name="zr_sb")
            t32a = pp.tile([32, 128], BF16, tag="t32a", name="t32a")

            nc.gpsimd.memset(ones_col[:], 1.0)
            nc.vector.memset(att1[:], 0.0)
            make_identity(nc, ident[:])

            nc.scalar.dma_start(qkb_t[:], qkb2[:])
            for i in range(KC):
                nc.sync.dma_start(qkw_t[i][:], qkw[i, :, :])
                nc.gpsimd.dma_start(vwtb_t[i][:], vwtb[i, :, :])

            pool2 = tc.alloc_tile_pool(name="pool2", bufs=1)
            vth2 = pool2.tile([128, NCH * 512], BF16, tag="vth2", name="vth2")
            att2 = pool2.tile([128, S], BF16, tag="att2", name="att2")
            nc.vector.memset(att2[:], 0.0)
            qkpool = tc.alloc_tile_pool(name="qkpool", bufs=1)
            q_sb = qkpool.tile([64, S], BF16, tag="q_sb", name="q_sb")
            k_sb = qkpool.tile([64, S], BF16, tag="k_sb", name="k_sb")
            ringA = tc.alloc_tile_pool(name="ringA", bufs=2)

            # ---- proj pass over x2: Q/K, V2, E_W (1 chunk behind), gathers ----
            psE2 = {}

            def emit_ew(jd):
                gd = jd // 4
                if jd % 4 == 0:
                    psE2[gd] = psA.tile([128, 512], F32, tag="ps", name="psE2")
                for hp in range(2):
                    sl = slice(128 * jd + 64 * hp, 128 * jd + 64 * hp + 64)
                    nc.tensor.matmul(
                        psE2[gd][64 * hp: 64 * hp + 64,
                                 128 * (jd % 4) + 64 * hp: 128 * (jd % 4) + 64 * hp + 64],
                        lhsT=k_sb[:, sl], rhs=q_sb[:, sl],
                        start=True, stop=True, skip_group_check=True,
                        tile_position=(0, 64 * hp),
                    )
                if jd % 4 == 3:
                    att2_g = att2[:].rearrange("p (j hq) -> p j hq", hq=128)
                    psE2_g = psE2[gd][:].rearrange("p (jl hq) -> p jl hq", hq=128)
                    for hp in range(2):
                        nc.scalar.activation(
                            out=att2_g[64 * hp: 64 * hp + 64, 4 * gd: 4 * gd + 4,
                                       64 * hp: 64 * hp + 64],
                            in_=psE2_g[64 * hp: 64 * hp + 64, :, 64 * hp: 64 * hp + 64],
                            func=mybir.ActivationFunctionType.Exp,
                        )

            for j in range(NCH):
                if j % 4 == 0:
                    x2c = ringA.tile([128, 4 * 512], BF16, tag="x2c", bufs=2, name="x2c")
                    if j == 0:
                        nc.scalar.dma_start(x2c[:, 0:512], x2p_v[:, 0:1, :])
                        nc.scalar.dma_start(x2c[:, 512:2048], x2p_v[:, 1:4, :])
                    else:
                        nc.sync.dma_start(x2c[:], x2p_v[:, j: j + 4, :])
                xc = x2c[:, 512 * (j % 4): 512 * (j % 4 + 1)]
                psQK = psB.tile([64, 256], F32, tag="psqk", name="psqk")
                for kc in range(KC):
                    nc.tensor.matmul(
                        psQK[:, 0:128],
                        lhsT=qkw_t[kc][:, 0:64],
                        rhs=xc[:, 128 * kc: 128 * (kc + 1)],
                        start=(kc == 0), stop=(kc == KC - 1),
                    )
                for kc in range(KC):
                    nc.tensor.matmul(
                        psQK[:, 128:256],
                        lhsT=qkw_t[kc][:, 64:128],
                        rhs=xc[:, 128 * kc: 128 * (kc + 1)],
                        start=(kc == 0), stop=(kc == KC - 1),
                    )
                # V2 projection (PE keeps running; E_W of previous chunk next)
                psV = psA.tile([128, 512], F32, tag="ps", name="psV2")
                for kc in range(KC):
                    nc.tensor.matmul(
                        psV[:],
                        lhsT=xc[:, 128 * kc: 128 * (kc + 1)],
                        rhs=vwtb_t[kc][:],
                        start=(kc == 0), stop=(kc == KC - 1),
                    )
                if j >= 1:
                    emit_ew(j - 1)
                nc.scalar.activation(
                    out=q_sb[:, 128 * j: 128 * (j + 1)], in_=psQK[:, 0:128],
                    func=mybir.ActivationFunctionType.Identity, bias=qkb_t[:, 0:1],
                )
                nc.scalar.activation(
                    out=k_sb[:, 128 * j: 128 * (j + 1)], in_=psQK[:, 128:256],
                    func=mybir.ActivationFunctionType.Identity, bias=qkb_t[:, 1:2],
                )
                nc.vector.tensor_copy(vth2[:, 512 * j: 512 * (j + 1)], psV[:])
            emit_ew(NCH - 1)

            # ---- E_H (64 mms) ----
            k_col = k_sb[:].rearrange("p (j hp em) -> p em (j hp)", hp=2, em=64)
            q_col = q_sb[:].rearrange("p (j hp em) -> p em (j hp)", hp=2, em=64)
            att1_g = att1[:].rearrange("p (j hq) -> p j hq", hq=128)
            for g8 in range(8):
                psE1 = psA.tile([128, 512], F32, tag="ps", name="psE1")
                psE1_g = psE1[:].rearrange("p (jl hq) -> p jl hq", hq=128)
                for wl in range(8):
                    w = 8 * g8 + wl
                    wp, j2l = w % 2, (w // 2) % 4
                    em = (w % 2) * 32 + w // 2
                    nc.tensor.matmul(
                        psE1[64 * wp: 64 * wp + 64,
                             128 * j2l + 64 * wp: 128 * j2l + 64 * wp + 64],
                        lhsT=k_col[:, em, :], rhs=q_col[:, em, :],
                        start=True, stop=True, skip_group_check=True,
                        tile_position=(0, 64 * wp),
                    )
                for wp in range(2):
                    nc.scalar.activation(
                        out=att1_g[64 * wp: 64 * wp + 64, 4 * g8: 4 * g8 + 4,
                                   64 * wp: 64 * wp + 64],
                        in_=psE1_g[64 * wp: 64 * wp + 64, :, 64 * wp: 64 * wp + 64],
                        func=mybir.ActivationFunctionType.Exp,
                    )

            ringA.release()
            qkpool.release()
            ringP = tc.alloc_tile_pool(name="ringP", bufs=2)

            # ---- pass-1 V1 pipeline (prologue emitted before normalize) ----
            PD = 24
            x1c_t = {}
            vt1_t = {}

            def emit_v1(jd):
                if jd % 4 == 0:
                    x1c_t[jd // 4] = ringP.tile(
                        [128, 4 * 512], BF16, tag="x1c", bufs=2, name="x1c"
                    )
                    nc.sync.dma_start(x1c_t[jd // 4][:], x1p_v[:, jd: jd + 4, :])
                xc = x1c_t[jd // 4][:, 512 * (jd % 4): 512 * (jd % 4 + 1)]
                psV = psA.tile([128, 512], F32, tag="ps", name="psV1")
                for kc in range(KC):
                    nc.tensor.matmul(
                        psV[:], lhsT=xc[:, 128 * kc: 128 * (kc + 1)],
                        rhs=vwtb_t[kc][:],
                        start=(kc == 0), stop=(kc == KC - 1),
                    )
                vt1 = ringP.tile([128, 512], BF16, tag="vt1", bufs=PD + 2, name="vt1")
                vt1_t[jd] = vt1
                nc.scalar.activation(out=vt1[:], in_=psV[:],
                                     func=mybir.ActivationFunctionType.Copy)

            # ---- colsum Z (att1 layout) -> recip -> transpose -> broadcast ----
            att2_zv = att2[:].rearrange(
                "p (j hp wp j2) -> p j2 wp (j hp)", hp=2, wp=2, j2=32
            )
            psZT = psA.tile([128, 512], F32, tag="ps", name="psZT")
            for j2 in range(NCH):
                nc.tensor.matmul(
                    psZT[:, j2: j2 + 1], lhsT=att1[:, 128 * j2: 128 * (j2 + 1)],
                    rhs=ones_col[:], start=True, stop=False, skip_group_check=True,
                )
                for wp in range(2):
                    nc.tensor.matmul(
                        psZT[64 * wp: 64 * wp + 64, j2: j2 + 1],
                        lhsT=att2_zv[:, j2, wp, :],
                        rhs=ones_col[:], start=False, stop=(wp == 1),
                        skip_group_check=True,
                    )
            with nc.allow_low_precision(reason="softmax recip in bf16"):
                nc.vector.reciprocal(zr_sb[:], psZT[:, 0:32])
            for jd in range(4):
                emit_v1(jd)
            psT1 = psA.tile([128, 1024], BF16, tag="ps", name="psT1")
            nc.tensor.transpose(psT1[0:32, 0:128], zr_sb[:], ident[:])
            nc.vector.tensor_copy(t32a[:], psT1[0:32, 0:128])
            rrow = partT1[0:1, 0:4096]
            nc.sync.dma_start(rr_d[:], t32a[:])
            nc.sync.dma_start(rrow, rr_d.rearrange("a b -> (a b)"))
            for jd in range(4, PD):
                emit_v1(jd)

            # normalize: one big partition broadcast, then per-chunk muls
            rfull = partT2[:, 0:4096]
            nc.gpsimd.partition_broadcast(rfull, rrow)
            rf_v = rfull.rearrange(
                "p (j2 wp j hp) -> p j hp wp j2", j2=32, wp=2, hp=2
            )
            for j in range(NCH):
                eng = nc.vector if j % 2 == 0 else nc.gpsimd
                eng.tensor_mul(
                    att2[:, 128 * j: 128 * (j + 1)],
                    att2[:, 128 * j: 128 * (j + 1)],
                    rf_v[:, j],
                )
            for n in range(8):
                cols = slice(512 * n, 512 * (n + 1))
                nc.vector.tensor_mul(att1[:, cols], att1[:, cols], rfull[:, cols])

            pT2_v = partT2[:].rearrange("p (cb s) -> p cb s", cb=4)
            pT1_v = partT1[:].rearrange("p (cb s) -> p cb s", cb=4)

            # ---- pass 1 main: att_W partials (V1 pipeline PD ahead) ----
            for j in range(NCH):
                if j + PD < NCH:
                    emit_v1(j + PD)
                att2c = att2[:, 128 * j: 128 * (j + 1)]
                psO2 = psA.tile([128, 512], F32, tag="ps", name="psO2")
                for cb in range(4):
                    nc.tensor.matmul(
                        psO2[:, 128 * cb: 128 * (cb + 1)],
                        lhsT=vth2[:, 512 * j + 128 * cb: 512 * j + 128 * (cb + 1)],
                        rhs=att2c, start=True, stop=True, skip_group_check=True,
                    )
                nc.vector.tensor_copy(pT2_v[:, :, 128 * j: 128 * (j + 1)], psO2[:])
                psO1 = psA.tile([128, 512], F32, tag="ps", name="psO1")
                for cb in range(4):
                    nc.tensor.matmul(
                        psO1[:, 128 * cb: 128 * (cb + 1)],
                        lhsT=vt1_t[j][:, 128 * cb: 128 * (cb + 1)],
                        rhs=att2c, start=True, stop=True, skip_group_check=True,
                    )
                nc.scalar.activation(out=pT1_v[:, :, 128 * j: 128 * (j + 1)], in_=psO1[:],
                                     func=mybir.ActivationFunctionType.Copy)

            ringP.release()
            pool2.release()
            ringQ = tc.alloc_tile_pool(name="ringQ", bufs=2)

            # ---- pass 2: att_H + combine; DVE does y2, ACT+gpsimd do y1 ----
            pT2_c = partT2[:].rearrange(
                "p (cb m wp j2) -> p j2 cb wp m", cb=4, m=64, wp=2, j2=32
            )
            pT1_c = partT1[:].rearrange(
                "p (cb m wp j2) -> p j2 cb wp m", cb=4, m=64, wp=2, j2=32
            )
            PDQ = 4
            x2wc_t, x1wc_t, vt2w_t, vt1w_t = {}, {}, {}, {}
            ys_t = {}

            def emit_vw(jd):
                if jd % 4 == 0:
                    x2wc_t[jd // 4] = ringQ.tile([128, 4 * 512], BF16, tag="x2wc",
                                                 bufs=2, name="x2wc")
                    nc.sync.dma_start(x2wc_t[jd // 4][:], x2pw_v[:, jd: jd + 4, :])
                    x1wc_t[jd // 4] = ringQ.tile([128, 4 * 512], BF16, tag="x1wc",
                                                 bufs=2, name="x1wc")
                    nc.sync.dma_start(x1wc_t[jd // 4][:], x1pw_v[:, jd: jd + 4, :])
                jl = jd % 4
                psVw2 = psA.tile([128, 512], F32, tag="ps", name="psVw2")
                for kc in range(KC):
                    nc.tensor.matmul(
                        psVw2[:],
                        lhsT=x2wc_t[jd // 4][:, 512 * jl + 128 * kc: 512 * jl + 128 * (kc + 1)],
                        rhs=vwtb_t[kc][:],
                        start=(kc == 0), stop=(kc == KC - 1),
                    )
                vt2w = ringQ.tile([128, 512], BF16, tag="vt2w", bufs=PDQ + 2, name="vt2w")
                vt2w_t[jd] = vt2w
                nc.scalar.activation(out=vt2w[:], in_=psVw2[:],
                                     func=mybir.ActivationFunctionType.Copy)
                psVw1 = psA.tile([128, 512], F32, tag="ps", name="psVw1")
                for kc in range(KC):
                    nc.tensor.matmul(
                        psVw1[:],
                        lhsT=x1wc_t[jd // 4][:, 512 * jl + 128 * kc: 512 * jl + 128 * (kc + 1)],
                        rhs=vwtb_t[kc][:],
                        start=(kc == 0), stop=(kc == KC - 1),
                    )
                vt1w = ringQ.tile([128, 512], BF16, tag="vt1w", bufs=PDQ + 2, name="vt1w")
                vt1w_t[jd] = vt1w
                nc.vector.tensor_copy(vt1w[:], psVw1[:])

            for jd in range(PDQ):
                emit_vw(jd)
            for j2 in range(NCH):
                gi, jj = j2 // 4, j2 % 4
                if j2 + PDQ < NCH:
                    emit_vw(j2 + PDQ)
                if jj == 0:
                    ys2 = ringQ.tile([128, 4 * 512], BF16, tag="ys2", bufs=2, name="ys2")
                    ys1 = ringQ.tile([128, 4 * 512], BF16, tag="ys1", bufs=2, name="ys1")
                att1c = att1[:, 128 * j2: 128 * (j2 + 1)]
                psF2 = psA.tile([128, 512], F32, tag="ps", name="psF2")
                for cb in range(4):
                    nc.tensor.matmul(
                        psF2[:, 128 * cb: 128 * (cb + 1)],
                        lhsT=vt2w_t[j2][:, 128 * cb: 128 * (cb + 1)],
                        rhs=att1c, start=True, stop=True, skip_group_check=True,
                    )
                psF1 = psA.tile([128, 512], F32, tag="ps", name="psF1")
                for cb in range(4):
                    nc.tensor.matmul(
                        psF1[:, 128 * cb: 128 * (cb + 1)],
                        lhsT=vt1w_t[j2][:, 128 * cb: 128 * (cb + 1)],
                        rhs=att1c, start=True, stop=True, skip_group_check=True,
                    )
                ys2_v = ys2[:].rearrange(
                    "p (cb jl wp m) -> p jl cb wp m", cb=4, jl=4, wp=2, m=64
                )
                ys1_v = ys1[:].rearrange(
                    "p (cb jl wp m) -> p jl cb wp m", cb=4, jl=4, wp=2, m=64
                )
                psF2_v = psF2[:].rearrange("p (cb wp m) -> p cb wp m", cb=4, wp=2, m=64)
                nc.vector.tensor_add(ys2_v[:, jj], psF2_v, pT2_c[:, j2])
                if j2 >= NCH - 2:
                    psF1_v = psF1[:].rearrange("p (cb wp m) -> p cb wp m", cb=4, wp=2, m=64)
                    nc.vector.tensor_add(ys1_v[:, jj], psF1_v, pT1_c[:, j2])
                else:
                    f1tmp = ringQ.tile([128, 512], BF16, tag="f1tmp", bufs=3, name="f1tmp")
                    nc.scalar.activation(out=f1tmp[:], in_=psF1[:],
                                         func=mybir.ActivationFunctionType.Copy)
                    f1_v = f1tmp[:].rearrange("p (cb wp m) -> p cb wp m", cb=4, wp=2, m=64)
                    nc.gpsimd.tensor_add(ys1_v[:, jj], f1_v, pT1_c[:, j2])
                if j2 < 24 and jj == 3:
                    nc.gpsimd.dma_start(y2t_v[gi], ys2[:])
                    nc.sync.dma_start(y1t_v[gi], ys1[:])
                elif j2 >= 24 and j2 % 2 == 1:
                    half = (jj - 1) // 2
                    y2h = y2t_v[gi].rearrange("p cb (hf sg) -> p hf cb sg", hf=2)
                    y1h = y1t_v[gi].rearrange("p cb (hf sg) -> p hf cb sg", hf=2)
                    ys2h = ys2[:].rearrange("p (cb hf sg) -> p hf cb sg", cb=4, hf=2)
                    ys1h = ys1[:].rearrange("p (cb hf sg) -> p hf cb sg", cb=4, hf=2)
                    nc.gpsimd.dma_start(y2h[:, half], ys2h[:, half])
                    nc.sync.dma_start(y1h[:, half], ys1h[:, half])

            ringQ.release()

    nc.compile()
    return nc


def make_in_maps(x2, x1, q_w, q_b, k_w, k_b, v_w, v_b, gamma):
    x2 = np.asarray(x2, dtype=np.float32)
    x1 = np.asarray(x1, dtype=np.float32)
    g = float(np.asarray(gamma).reshape(-1)[0])
    bf16 = ml_dtypes.bfloat16
    qkw = (
        np.concatenate([np.asarray(q_w).T, np.asarray(k_w).T], axis=1)
        .reshape(KC, 128, 128).astype(bf16)
    )
    qkb2 = np.stack([np.asarray(q_b), np.asarray(k_b)], axis=1).astype(bf16)
    vwtb = (g * np.asarray(v_w)).T.reshape(KC, 128, C).astype(bf16)

    def pack_p(xfl):
        t = xfl.reshape(KC, 128, NCH, 2, 32, 2)  # kc ch j hp u wpar
        return np.ascontiguousarray(
            t.transpose(2, 1, 0, 3, 5, 4).reshape(NCH, 128, KC * 128).astype(bf16)
        )

    def pack_pw(xfl):
        t = xfl.reshape(KC, 128, 64, 32, 2)  # kc ch h j2 wq
        return np.ascontiguousarray(
            t.transpose(3, 1, 0, 4, 2).reshape(NCH, 128, KC * 128).astype(bf16)
        )

    in_maps = []
    for b in range(B):
        x2fl = x2[b].reshape(C, S)
        x1fl = x1[b].reshape(C, S)
        in_maps.append(
            {
                "x2p": pack_p(x2fl),
                "x1p": pack_p(x1fl),
                "x2pw": pack_pw(x2fl),
                "x1pw": pack_pw(x1fl),
                "qkw": qkw,
                "qkb2": qkb2,
                "vwtb": vwtb,
            }
        )
    return in_maps, g


def assemble_outputs(res, x2, x1, v_b, g):
    y2 = np.empty((B, C, H, W), np.float32)
    y1 = np.empty((B, C, H, W), np.float32)
    gvb = (g * np.asarray(v_b, dtype=np.float32))[None, :, None, None]
    for b in range(B):
        y2[b] = unpermute(np.asarray(res[b]["y2t"]))
        y1[b] = unpermute(np.asarray(res[b]["y1t"]))
    y2 += gvb
    y2 += np.asarray(x2, dtype=np.float32)
    y1 += gvb
    y1 += np.asarray(x1, dtype=np.float32)
    return y2, y1


def unpermute(yt):
    # yt [C, s''=j2*128+wp*64+h] -> y[c, h, w=2*j2+wp]
    return np.ascontiguousarray(
        yt.astype(np.float32).reshape(C, 32, 2, 64).transpose(0, 3, 1, 2).reshape(C, H, W)
    )


def kernel(x2, x1, q_w, q_b, k_w, k_b, v_w, v_b, gamma):
    in_maps, g = make_in_maps(x2, x1, q_w, q_b, k_w, k_b, v_w, v_b, gamma)
    if "nc" not in _CACHED:
        _CACHED["nc"] = build_nc()
    nc = _CACHED["nc"]
    res = run_bass_kernel_spmd(nc, in_maps, list(range(B))).results
    return assemble_outputs(res, x2, x1, v_b, g)


# revision 27
# speedup vs baseline: 1.2694x; 1.0045x over previous
"""Criss-cross attention (CC module) Trainium2 Bass kernel, v5 (c-major).

v5 vs v4: pass-2 V comes from SBUF gathers of vth2/vt1 (no x?pw recompute);
colsum Z computed as [128,32] per chunk layout via N=1 matmuls, reciprocal
on [128,64] (full DVE lanes), PE-transposed to [32,128] rows, broadcast back
via all-ones matmul; combines are single [128,512] DVE ops using the merged
(cb, wp, h) access pattern on partT.

Index maps (per chunk j of 128 spatial positions):
  h-major chunk j: col = kc*128 + hp*64 + em, em=(w%2)*32+w//2, h=2j+hp
  att1 chunk j2 quadrant (wp,wp): [h' plain, h plain] for w=2*j2+wp
  att2 chunk j  quadrant (hp,hp): [W' em, w em] for h=2j+hp
  vtw row wq*64+h, col j2*512+c = v[c, h, 2*j2+wq]
  partT col = cb*4096 + j*128 + hp*64 + em ; y2t col s'' = j2*128 + wp*64 + h
"""

import numpy as np
import ml_dtypes

import concourse.bass as bass
import concourse.mybir as mybir
import concourse.tile as tile
from concourse import bacc
from concourse.bass_utils import run_bass_kernel_spmd
from concourse.masks import make_identity

BF16 = mybir.dt.bfloat16
F32 = mybir.dt.float32

B, C, H, W = 8, 512, 64, 64
CQ = 64
S = H * W  # 4096
NCH = S // 128  # 32 spatial chunks of 128
KC = C // 128  # 4 contraction chunks

_CACHED = {}


def build_nc():
    nc = bacc.Bacc("TRN2", target_bir_lowering=False, debug=False)

    x2p = nc.dram_tensor("x2p", [NCH, 128, 512], BF16, kind="ExternalInput")
    x1p = nc.dram_tensor("x1p", [NCH, 128, 512], BF16, kind="ExternalInput")
    x2pw = nc.dram_tensor("x2pw", [NCH, 128, 512], BF16, kind="ExternalInput")
    x1pw = nc.dram_tensor("x1pw", [NCH, 128, 512], BF16, kind="ExternalInput")
    qkw = nc.dram_tensor("qkw", [KC, 128, 128], BF16, kind="ExternalInput")
    qkb2 = nc.dram_tensor("qkb2", [64, 2], BF16, kind="ExternalInput")
    vwtb = nc.dram_tensor("vwtb", [KC, 128, C], BF16, kind="ExternalInput")

    rr_d = nc.dram_tensor("rr_d", [32, 128], BF16)
    y2t = nc.dram_tensor("y2t", [C, S], BF16, kind="ExternalOutput")
    y1t = nc.dram_tensor("y1t", [C, S], BF16, kind="ExternalOutput")

    x2p_v = x2p.rearrange("j p c -> p j c")
    x1p_v = x1p.rearrange("j p c -> p j c")
    x2pw_v = x2pw.rearrange("j p c -> p j c")
    x1pw_v = x1pw.rearrange("j p c -> p j c")
    y2t_v = y2t.rearrange("(cb p) (gi sg) -> gi p cb sg", p=128, sg=512)
    y1t_v = y1t.rearrange("(cb p) (gi sg) -> gi p cb sg", p=128, sg=512)

    with tile.TileContext(nc) as tc:
        with (
            tc.tile_pool(name="persist", bufs=1) as pp,
            tc.tile_pool(name="psA", bufs=6, space="PSUM") as psA,
            tc.tile_pool(name="psB", bufs=2, space="PSUM") as psB,
        ):
            # ---- persistent tiles ----
            qkw_t = [pp.tile([128, 128], BF16, tag=f"qkw_{i}", name=f"qkw_{i}") for i in range(KC)]
            vwtb_t = [pp.tile([128, C], BF16, tag=f"vwtb_{i}", name=f"vwtb_{i}") for i in range(KC)]
            qkb_t = pp.tile([64, 2], BF16, tag="qkb", name="qkb")
            ones_col = pp.tile([128, 1], BF16, tag="ones_col", name="ones_col")
            ident = pp.tile([128, 128], BF16, tag="ident", name="ident")
            att1 = pp.tile([128, S], BF16, tag="att1", name="att1")
            partT2 = pp.tile([128, 4 * S], BF16, tag="partT2", name="partT2")
            partT1 = pp.tile([128, 4 * S], BF16, tag="partT1", name="partT1")
            zr_sb = pp.tile([128, 32], BF16, tag="zr_sb", name="zr_sb")
            t32a = pp.tile([32, 128], BF16, tag="t32a", name="t32a")

            nc.gpsimd.memset(ones_col[:], 1.0)
            nc.vector.memset(att1[:], 0.0)
            make_identity(nc, ident[:])

            for i in range(KC):
                nc.sync.dma_start(qkw_t[i][:], qkw[i, :, :])
                nc.gpsimd.dma_start(vwtb_t[i][:], vwtb[i, :, :])
            nc.gpsimd.dma_start(qkb_t[:], qkb2[:])

            pool2 = tc.alloc_tile_pool(name="pool2", bufs=1)
            vth2 = pool2.tile([128, NCH * 512], BF16, tag="vth2", name="vth2")
            att2 = pool2.tile([128, S], BF16, tag="att2", name="att2")
            nc.vector.memset(att2[:], 0.0)
            qkpool = tc.alloc_tile_pool(name="qkpool", bufs=1)
            q_sb = qkpool.tile([64, S], BF16, tag="q_sb", name="q_sb")
            k_sb = qkpool.tile([64, S], BF16, tag="k_sb", name="k_sb")
            ringA = tc.alloc_tile_pool(name="ringA", bufs=2)

            # ---- proj pass over x2: Q/K, V2, E_W (1 chunk behind), gathers ----
            psE2 = {}

            def emit_ew(jd):
                gd = jd // 4
                if jd % 4 == 0:
                    psE2[gd] = psA.tile([128, 512], F32, tag="ps", name="psE2")
                for hp in range(2):
                    sl = slice(128 * jd + 64 * hp, 128 * jd + 64 * hp + 64)
                    nc.tensor.matmul(
                        psE2[gd][64 * hp: 64 * hp + 64,
                                 128 * (jd % 4) + 64 * hp: 128 * (jd % 4) + 64 * hp + 64],
                        lhsT=k_sb[:, sl], rhs=q_sb[:, sl],
                        start=True, stop=True, skip_group_check=True,
                        tile_position=(0, 64 * hp),
                    )
                if jd % 4 == 3:
                    att2_g = att2[:].rearrange("p (j hq) -> p j hq", hq=128)
                    psE2_g = psE2[gd][:].rearrange("p (jl hq) -> p jl hq", hq=128)
                    for hp in range(2):
                        nc.scalar.activation(
                            out=att2_g[64 * hp: 64 * hp + 64, 4 * gd: 4 * gd + 4,
                                       64 * hp: 64 * hp + 64],
                            in_=psE2_g[64 * hp: 64 * hp + 64, :, 64 * hp: 64 * hp + 64],
                            func=mybir.ActivationFunctionType.Exp,
                        )

            for j in range(NCH):
                if j % 4 == 0:
                    x2c = ringA.tile([128, 4 * 512], BF16, tag="x2c", bufs=2, name="x2c")
                    if j == 0:
                        nc.scalar.dma_start(x2c[:, 0:512], x2p_v[:, 0:1, :])
                        nc.scalar.dma_start(x2c[:, 512:2048], x2p_v[:, 1:4, :])
                    else:
                        nc.sync.dma_start(x2c[:], x2p_v[:, j: j + 4, :])
                xc = x2c[:, 512 * (j % 4): 512 * (j % 4 + 1)]
                psQK = psB.tile([64, 256], F32, tag="psqk", name="psqk")
                for kc in range(KC):
                    nc.tensor.matmul(
                        psQK[:, 0:128],
                        lhsT=qkw_t[kc][:, 0:64],
                        rhs=xc[:, 128 * kc: 128 * (kc + 1)],
                        start=(kc == 0), stop=(kc == KC - 1),
                    )
                for kc in range(KC):
                    nc.tensor.matmul(
                        psQK[:, 128:256],
                        lhsT=qkw_t[kc][:, 64:128],
                        rhs=xc[:, 128 * kc: 128 * (kc + 1)],
                        start=(kc == 0), stop=(kc == KC - 1),
                    )
                # V2 projection (PE keeps running; E_W of previous chunk next)
                psV = psA.tile([128, 512], F32, tag="ps", name="psV2")
                for kc in range(KC):
                    nc.tensor.matmul(
                        psV[:],
                        lhsT=xc[:, 128 * kc: 128 * (kc + 1)],
                        rhs=vwtb_t[kc][:],
                        start=(kc == 0), stop=(kc == KC - 1),
                    )
                if j >= 1:
                    emit_ew(j - 1)
                nc.scalar.activation(
                    out=q_sb[:, 128 * j: 128 * (j + 1)], in_=psQK[:, 0:128],
                    func=mybir.ActivationFunctionType.Identity, bias=qkb_t[:, 0:1],
                )
                nc.scalar.activation(
                    out=k_sb[:, 128 * j: 128 * (j + 1)], in_=psQK[:, 128:256],
                    func=mybir.ActivationFunctionType.Identity, bias=qkb_t[:, 1:2],
                )
                nc.vector.tensor_copy(vth2[:, 512 * j: 512 * (j + 1)], psV[:])
            emit_ew(NCH - 1)

            # ---- E_H (64 mms) ----
            k_col = k_sb[:].rearrange("p (j hp em) -> p em (j hp)", hp=2, em=64)
            q_col = q_sb[:].rearrange("p (j hp em) -> p em (j hp)", hp=2, em=64)
            att1_g = att1[:].rearrange("p (j hq) -> p j hq", hq=128)
            for g8 in range(8):
                psE1 = psA.tile([128, 512], F32, tag="ps", name="psE1")
                psE1_g = psE1[:].rearrange("p (jl hq) -> p jl hq", hq=128)
                for wl in range(8):
                    w = 8 * g8 + wl
                    wp, j2l = w % 2, (w // 2) % 4
                    em = (w % 2) * 32 + w // 2
                    nc.tensor.matmul(
                        psE1[64 * wp: 64 * wp + 64,
                             128 * j2l + 64 * wp: 128 * j2l + 64 * wp + 64],
                        lhsT=k_col[:, em, :], rhs=q_col[:, em, :],
                        start=True, stop=True, skip_group_check=True,
                        tile_position=(0, 64 * wp),
                    )
                for wp in range(2):
                    nc.scalar.activation(
                        out=att1_g[64 * wp: 64 * wp + 64, 4 * g8: 4 * g8 + 4,
                                   64 * wp: 64 * wp + 64],
                        in_=psE1_g[64 * wp: 64 * wp + 64, :, 64 * wp: 64 * wp + 64],
                        func=mybir.ActivationFunctionType.Exp,
                    )

            ringA.release()
            qkpool.release()
            ringP = tc.alloc_tile_pool(name="ringP", bufs=2)

            # ---- pass-1 V1 pipeline (prologue emitted before normalize) ----
            PD = 24
            x1c_t = {}
            vt1_t = {}

            def emit_v1(jd):
                if jd % 4 == 0:
                    x1c_t[jd // 4] = ringP.tile(
                        [128, 4 * 512], BF16, tag="x1c", bufs=2, name="x1c"
                    )
                    nc.sync.dma_start(x1c_t[jd // 4][:], x1p_v[:, jd: jd + 4, :])
                xc = x1c_t[jd // 4][:, 512 * (jd % 4): 512 * (jd % 4 + 1)]
                psV = psA.tile([128, 512], F32, tag="ps", name="psV1")
                for kc in range(KC):
                    nc.tensor.matmul(
                        psV[:], lhsT=xc[:, 128 * kc: 128 * (kc + 1)],
                        rhs=vwtb_t[kc][:],
                        start=(kc == 0), stop=(kc == KC - 1),
                    )
                vt1 = ringP.tile([128, 512], BF16, tag="vt1", bufs=PD + 2, name="vt1")
                vt1_t[jd] = vt1
                nc.scalar.activation(out=vt1[:], in_=psV[:],
                                     func=mybir.ActivationFunctionType.Copy)

            # ---- colsum Z (att1 layout) -> recip -> transpose -> broadcast ----
            att2_zv = att2[:].rearrange(
                "p (j hp wp j2) -> p j2 wp (j hp)", hp=2, wp=2, j2=32
            )
            psZT = psA.tile([128, 512], F32, tag="ps", name="psZT")
            for j2 in range(NCH):
                nc.tensor.matmul(
                    psZT[:, j2: j2 + 1], lhsT=att1[:, 128 * j2: 128 * (j2 + 1)],
                    rhs=ones_col[:], start=True, stop=False, skip_group_check=True,
                )
                for wp in range(2):
                    nc.tensor.matmul(
                        psZT[64 * wp: 64 * wp + 64, j2: j2 + 1],
                        lhsT=att2_zv[:, j2, wp, :],
                        rhs=ones_col[:], start=False, stop=(wp == 1),
                        skip_group_check=True,
                    )
            with nc.allow_low_precision(reason="softmax recip in bf16"):
                nc.vector.reciprocal(zr_sb[:], psZT[:, 0:32])
            for jd in range(4):
                emit_v1(jd)
            psT1 = psA.tile([128, 1024], BF16, tag="ps", name="psT1")
            nc.tensor.transpose(psT1[0:32, 0:128], zr_sb[:], ident[:])
            nc.vector.tensor_copy(t32a[:], psT1[0:32, 0:128])
            rrow = partT1[0:1, 0:4096]
            nc.sync.dma_start(rr_d[:], t32a[:])
            nc.sync.dma_start(rrow, rr_d.rearrange("a b -> (a b)"))
            for jd in range(4, PD):
                emit_v1(jd)

            # normalize: one big partition broadcast, then per-chunk muls
            rfull = partT2[:, 0:4096]
            nc.gpsimd.partition_broadcast(rfull, rrow)
            rf_v = rfull.rearrange(
                "p (j2 wp j hp) -> p j hp wp j2", j2=32, wp=2, hp=2
            )
            for j in range(NCH):
                eng = nc.vector if j % 2 == 0 else nc.gpsimd
                eng.tensor_mul(
                    att2[:, 128 * j: 128 * (j + 1)],
                    att2[:, 128 * j: 128 * (j + 1)],
                    rf_v[:, j],
                )
            for n in range(8):
                cols = slice(512 * n, 512 * (n + 1))
                nc.vector.tensor_mul(att1[:, cols], att1[:, cols], rfull[:, cols])

            pT2_v = partT2[:].rearrange("p (cb s) -> p cb s", cb=4)
            pT1_v = partT1[:].rearrange("p (cb s) -> p cb s", cb=4)

            # ---- pass 1 main: att_W partials (V1 pipeline PD ahead) ----
            for j in range(NCH):
                if j + PD < NCH:
                    emit_v1(j + PD)
                att2c = att2[:, 128 * j: 128 * (j + 1)]
                psO2 = psA.tile([128, 512], F32, tag="ps", name="psO2")
                for cb in range(4):
                    nc.tensor.matmul(
                        psO2[:, 128 * cb: 128 * (cb + 1)],
                        lhsT=vth2[:, 512 * j + 128 * cb: 512 * j + 128 * (cb + 1)],
                        rhs=att2c, start=True, stop=True, skip_group_check=True,
                    )
                nc.vector.tensor_copy(pT2_v[:, :, 128 * j: 128 * (j + 1)], psO2[:])
                psO1 = psA.tile([128, 512], F32, tag="ps", name="psO1")
                for cb in range(4):
                    nc.tensor.matmul(
                        psO1[:, 128 * cb: 128 * (cb + 1)],
                        lhsT=vt1_t[j][:, 128 * cb: 128 * (cb + 1)],
                        rhs=att2c, start=True, stop=True, skip_group_check=True,
                    )
                nc.scalar.activation(out=pT1_v[:, :, 128 * j: 128 * (j + 1)], in_=psO1[:],
                                     func=mybir.ActivationFunctionType.Copy)

            ringP.release()
            pool2.release()
            ringQ = tc.alloc_tile_pool(name="ringQ", bufs=2)

            # ---- pass 2: att_H + combine; DVE does y2, ACT+gpsimd do y1 ----
            pT2_c = partT2[:].rearrange(
                "p (cb m wp j2) -> p j2 cb wp m", cb=4, m=64, wp=2, j2=32
            )
            pT1_c = partT1[:].rearrange(
                "p (cb m wp j2) -> p j2 cb wp m", cb=4, m=64, wp=2, j2=32
            )
            PDQ = 4
            x2wc_t, x1wc_t, vt2w_t, vt1w_t = {}, {}, {}, {}
            ys_t = {}

            def emit_vw(jd):
                if jd % 4 == 0:
                    x2wc_t[jd // 4] = ringQ.tile([128, 4 * 512], BF16, tag="x2wc",
                                                 bufs=2, name="x2wc")
                    x1wc_t[jd // 4] = ringQ.tile([128, 4 * 512], BF16, tag="x1wc",
                                                 bufs=2, name="x1wc")
                    if jd == 0:
                        nc.sync.dma_start(x2wc_t[0][:, 0:512], x2pw_v[:, 0:1, :])
                        nc.scalar.dma_start(x1wc_t[0][:, 0:512], x1pw_v[:, 0:1, :])
                        nc.sync.dma_start(x2wc_t[0][:, 512:2048], x2pw_v[:, 1:4, :])
                        nc.scalar.dma_start(x1wc_t[0][:, 512:2048], x1pw_v[:, 1:4, :])
                    else:
                        nc.sync.dma_start(x2wc_t[jd // 4][:], x2pw_v[:, jd: jd + 4, :])
                        nc.sync.dma_start(x1wc_t[jd // 4][:], x1pw_v[:, jd: jd + 4, :])
                jl = jd % 4
                psVw2 = psA.tile([128, 512], F32, tag="ps", name="psVw2")
                for kc in range(KC):
                    nc.tensor.matmul(
                        psVw2[:],
                        lhsT=x2wc_t[jd // 4][:, 512 * jl + 128 * kc: 512 * jl + 128 * (kc + 1)],
                        rhs=vwtb_t[kc][:],
                        start=(kc == 0), stop=(kc == KC - 1),
                    )
                vt2w = ringQ.tile([128, 512], BF16, tag="vt2w", bufs=PDQ + 2, name="vt2w")
                vt2w_t[jd] = vt2w
                nc.scalar.activation(out=vt2w[:], in_=psVw2[:],
                                     func=mybir.ActivationFunctionType.Copy)
                psVw1 = psA.tile([128, 512], F32, tag="ps", name="psVw1")
                for kc in range(KC):
                    nc.tensor.matmul(
                        psVw1[:],
                        lhsT=x1wc_t[jd // 4][:, 512 * jl + 128 * kc: 512 * jl + 128 * (kc + 1)],
                        rhs=vwtb_t[kc][:],
                        start=(kc == 0), stop=(kc == KC - 1),
                    )
                vt1w = ringQ.tile([128, 512], BF16, tag="vt1w", bufs=PDQ + 2, name="vt1w")
                vt1w_t[jd] = vt1w
                nc.vector.tensor_copy(vt1w[:], psVw1[:])

            for jd in range(PDQ):
                emit_vw(jd)
            for j2 in range(NCH):
                gi, jj = j2 // 4, j2 % 4
                if j2 + PDQ < NCH:
                    emit_vw(j2 + PDQ)
                if jj == 0:
                    ys2 = ringQ.tile([128, 4 * 512], BF16, tag="ys2", bufs=2, name="ys2")
                    ys1 = ringQ.tile([128, 4 * 512], BF16, tag="ys1", bufs=2, name="ys1")
                att1c = att1[:, 128 * j2: 128 * (j2 + 1)]
                psF2 = psA.tile([128, 512], F32, tag="ps", name="psF2")
                for cb in range(4):
                    nc.tensor.matmul(
                        psF2[:, 128 * cb: 128 * (cb + 1)],
                        lhsT=vt2w_t[j2][:, 128 * cb: 128 * (cb + 1)],
                        rhs=att1c, start=True, stop=True, skip_group_check=True,
                    )
                psF1 = psA.tile([128, 512], F32, tag="ps", name="psF1")
                for cb in range(4):
                    nc.tensor.matmul(
                        psF1[:, 128 * cb: 128 * (cb + 1)],
                        lhsT=vt1w_t[j2][:, 128 * cb: 128 * (cb + 1)],
                        rhs=att1c, start=True, stop=True, skip_group_check=True,
                    )
                ys2_v = ys2[:].rearrange(
                    "p (cb jl wp m) -> p jl cb wp m", cb=4, jl=4, wp=2, m=64
                )
                ys1_v = ys1[:].rearrange(
                    "p (cb jl wp m) -> p jl cb wp m", cb=4, jl=4, wp=2, m=64
                )
                psF2_v = psF2[:].rearrange("p (cb wp m) -> p cb wp m", cb=4, wp=2, m=64)
                psF1_v = psF1[:].rearrange("p (cb wp m) -> p cb wp m", cb=4, wp=2, m=64)
                if j2 >= NCH - 4:
                    f2tmp = ringQ.tile([128, 512], BF16, tag="f1tmp", bufs=3, name="f2tmp")
                    nc.scalar.activation(out=f2tmp[:], in_=psF2[:],
                                         func=mybir.ActivationFunctionType.Copy)
                    f2_v = f2tmp[:].rearrange("p (cb wp m) -> p cb wp m", cb=4, wp=2, m=64)
                    nc.gpsimd.tensor_add(ys2_v[:, jj], f2_v, pT2_c[:, j2])
                    nc.vector.tensor_add(ys1_v[:, jj], psF1_v, pT1_c[:, j2])
                else:
                    nc.vector.tensor_add(ys2_v[:, jj], psF2_v, pT2_c[:, j2])
                    f1tmp = ringQ.tile([128, 512], BF16, tag="f1tmp", bufs=3, name="f1tmp")
                    nc.scalar.activation(out=f1tmp[:], in_=psF1[:],
                                         func=mybir.ActivationFunctionType.Copy)
                    f1_v = f1tmp[:].rearrange("p (cb wp m) -> p cb wp m", cb=4, wp=2, m=64)
                    nc.gpsimd.tensor_add(ys1_v[:, jj], f1_v, pT1_c[:, j2])
                if j2 < 24 and jj == 3:
                    nc.gpsimd.dma_start(y2t_v[gi], ys2[:])
                    nc.sync.dma_start(y1t_v[gi], ys1[:])
                elif j2 >= 24 and j2 % 2 == 1:
                    half = (jj - 1) // 2
                    y2h = y2t_v[gi].rearrange("p cb (hf sg) -> p hf cb sg", hf=2)
                    y1h = y1t_v[gi].rearrange("p cb (hf sg) -> p hf cb sg", hf=2)
                    ys2h = ys2[:].rearrange("p (cb hf sg) -> p hf cb sg", cb=4, hf=2)
                    ys1h = ys1[:].rearrange("p (cb hf sg) -> p hf cb sg", cb=4, hf=2)
                    nc.gpsimd.dma_start(y2h[:, half], ys2h[:, half])
                    nc.sync.dma_start(y1h[:, half], ys1h[:, half])

            ringQ.release()

    nc.compile()
    return nc


def make_in_maps(x2, x1, q_w, q_b, k_w, k_b, v_w, v_b, gamma):
    x2 = np.asarray(x2, dtype=np.float32)
    x1 = np.asarray(x1, dtype=np.float32)
    g = float(np.asarray(gamma).reshape(-1)[0])
    bf16 = ml_dtypes.bfloat16
    qkw = (
        np.concatenate([np.asarray(q_w).T, np.asarray(k_w).T], axis=1)
        .reshape(KC, 128, 128).astype(bf16)
    )
    qkb2 = np.stack([np.asarray(q_b), np.asarray(k_b)], axis=1).astype(bf16)
    vwtb = (g * np.asarray(v_w)).T.reshape(KC, 128, C).astype(bf16)

    def pack_p(xfl):
        t = xfl.reshape(KC, 128, NCH, 2, 32, 2)  # kc ch j hp u wpar
        return np.ascontiguousarray(
            t.transpose(2, 1, 0, 3, 5, 4).reshape(NCH, 128, KC * 128).astype(bf16)
        )

    def pack_pw(xfl):
        t = xfl.reshape(KC, 128, 64, 32, 2)  # kc ch h j2 wq
        return np.ascontiguousarray(
            t.transpose(3, 1, 0, 4, 2).reshape(NCH, 128, KC * 128).astype(bf16)
        )

    in_maps = []
    for b in range(B):
        x2fl = x2[b].reshape(C, S)
        x1fl = x1[b].reshape(C, S)
        in_maps.append(
            {
                "x2p": pack_p(x2fl),
                "x1p": pack_p(x1fl),
                "x2pw": pack_pw(x2fl),
                "x1pw": pack_pw(x1fl),
                "qkw": qkw,
                "qkb2": qkb2,
                "vwtb": vwtb,
            }
        )
    return in_maps, g


def assemble_outputs(res, x2, x1, v_b, g):
    y2 = np.empty((B, C, H, W), np.float32)
    y1 = np.empty((B, C, H, W), np.float32)
    gvb = (g * np.asarray(v_b, dtype=np.float32))[None, :, None, None]
    for b in range(B):
        y2[b] = unpermute(np.asarray(res[b]["y2t"]))
        y1[b] = unpermute(np.asarray(res[b]["y1t"]))
    y2 += gvb
    y2 += np.asarray(x2, dtype=np.float32)
    y1 += gvb
    y1 += np.asarray(x1, dtype=np.float32)
    return y2, y1


def unpermute(yt):
    # yt [C, s''=j2*128+wp*64+h] -> y[c, h, w=2*j2+wp]
    return np.ascontiguousarray(
        yt.astype(np.float32).reshape(C, 32, 2, 64).transpose(0, 3, 1, 2).reshape(C, H, W)
    )


def kernel(x2, x1, q_w, q_b, k_w, k_b, v_w, v_b, gamma):
    in_maps, g = make_in_maps(x2, x1, q_w, q_b, k_w, k_b, v_w, v_b, gamma)
    if "nc" not in _CACHED:
        _CACHED["nc"] = build_nc()
    nc = _CACHED["nc"]
    res = run_bass_kernel_spmd(nc, in_maps, list(range(B))).results
    return assemble_outputs(res, x2, x1, v_b, g)


# revision 30
# speedup vs baseline: 1.2771x; 1.0061x over previous
"""Criss-cross attention (CC module) Trainium2 Bass kernel, final (c-major).

Distribution: data-parallel over batch B=8 across the 8 NeuronCores (one
batch element per core), full inputs sharded/gathered on the host.

Design: all attention outputs are computed CHANNEL-major ([C, S]) by using V
as the stationary matmul operand, so the h-major <-> w-major partial regroup
is a pure column access pattern (no partition crossing, no DRAM round trip,
no SBUF-to-SBUF scatter). x is host-packed in both h-major (x?p) and w-major
(x?pw) chunk orders; pass 2 recomputes V from x?pw (cheaper than gathers).
gamma is folded into vwtb host-side; the residual x and g*v_b bias (softmax
rows sum to 1) are added on the host. Softmax colsums are computed [128,32]
via N=1 matmuls, reciprocal uses all DVE lanes, and the r-row is replicated
across partitions with one gpsimd partition_broadcast after a tiny DRAM
bounce. Matmul emission is software-pipelined (E_W one chunk behind Q/K,
V1/Vw projections PD/PDQ chunks ahead of their consumers) so the PE never
waits on ACT drains; elementwise work is spread across ACT/DVE/GpSimd.

Index maps (per chunk j of 128 spatial positions):
  h-major chunk j: col = kc*128 + hp*64 + em, em=(w%2)*32+w//2, h=2j+hp
  att1 chunk j2 quadrant (wp,wp): [h' plain, h plain] for w=2*j2+wp
  att2 chunk j  quadrant (hp,hp): [W' em, w em] for h=2j+hp
  partT col = cb*4096 + j*128 + hp*64 + em ; y2t col s'' = j2*128 + wp*64 + h
"""

import numpy as np
import ml_dtypes

import concourse.bass as bass
import concourse.mybir as mybir
import concourse.tile as tile
from concourse import bacc
from concourse.bass_utils import run_bass_kernel_spmd
from concourse.masks import make_identity

BF16 = mybir.dt.bfloat16
F32 = mybir.dt.float32

B, C, H, W = 8, 512, 64, 64
CQ = 64
S = H * W  # 4096
NCH = S // 128  # 32 spatial chunks of 128
KC = C // 128  # 4 contraction chunks

_CACHED = {}


def build_nc():
    nc = bacc.Bacc("TRN2", target_bir_lowering=False, debug=False)

    x2p = nc.dram_tensor("x2p", [NCH, 128, 512], BF16, kind="ExternalInput")
    x1p = nc.dram_tensor("x1p", [NCH, 128, 512], BF16, kind="ExternalInput")
    x2pw = nc.dram_tensor("x2pw", [NCH, 128, 512], BF16, kind="ExternalInput")
    x1pw = nc.dram_tensor("x1pw", [NCH, 128, 512], BF16, kind="ExternalInput")
    qkw = nc.dram_tensor("qkw", [KC, 128, 128], BF16, kind="ExternalInput")
    qkb2 = nc.dram_tensor("qkb2", [64, 2], BF16, kind="ExternalInput")
    vwtb = nc.dram_tensor("vwtb", [KC, 128, C], BF16, kind="ExternalInput")

    rr_d = nc.dram_tensor("rr_d", [32, 128], BF16)
    y2t = nc.dram_tensor("y2t", [C, S], BF16, kind="ExternalOutput")
    y1t = nc.dram_tensor("y1t", [C, S], BF16, kind="ExternalOutput")

    x2p_v = x2p.rearrange("j p c -> p j c")
    x1p_v = x1p.rearrange("j p c -> p j c")
    x2pw_v = x2pw.rearrange("j p c -> p j c")
    x1pw_v = x1pw.rearrange("j p c -> p j c")
    y2t_v = y2t.rearrange("(cb p) (gi sg) -> gi p cb sg", p=128, sg=512)
    y1t_v = y1t.rearrange("(cb p) (gi sg) -> gi p cb sg", p=128, sg=512)

    with tile.TileContext(nc) as tc:
        with (
            tc.tile_pool(name="persist", bufs=1) as pp,
            tc.tile_pool(name="psA", bufs=6, space="PSUM") as psA,
            tc.tile_pool(name="psB", bufs=2, space="PSUM") as psB,
        ):
            # ---- persistent tiles ----
            qkw_t = [pp.tile([128, 128], BF16, tag=f"qkw_{i}", name=f"qkw_{i}") for i in range(KC)]
            vwtb_t = [pp.tile([128, C], BF16, tag=f"vwtb_{i}", name=f"vwtb_{i}") for i in range(KC)]
            qkb_t = pp.tile([64, 2], BF16, tag="qkb", name="qkb")
            ones_col = pp.tile([128, 1], BF16, tag="ones_col", name="ones_col")
            ident = pp.tile([128, 128], BF16, tag="ident", name="ident")
            att1 = pp.tile([128, S], BF16, tag="att1", name="att1")
            partT2 = pp.tile([128, 4 * S], BF16, tag="partT2", name="partT2")
            partT1 = pp.tile([128, 4 * S], BF16, tag="partT1", name="partT1")
            zr_sb = pp.tile([128, 32], BF16, tag="zr_sb", name="zr_sb")
            t32a = pp.tile([32, 128], BF16, tag="t32a", name="t32a")

            nc.gpsimd.memset(ones_col[:], 1.0)
            nc.vector.memset(att1[:], 0.0)
            make_identity(nc, ident[:])

            for i in range(KC):
                nc.sync.dma_start(qkw_t[i][:], qkw[i, :, :])
                nc.gpsimd.dma_start(vwtb_t[i][:], vwtb[i, :, :])
            nc.gpsimd.dma_start(qkb_t[:], qkb2[:])

            pool2 = tc.alloc_tile_pool(name="pool2", bufs=1)
            vth2 = pool2.tile([128, NCH * 512], BF16, tag="vth2", name="vth2")
            att2 = pool2.tile([128, S], BF16, tag="att2", name="att2")
            nc.vector.memset(att2[:], 0.0)
            qkpool = tc.alloc_tile_pool(name="qkpool", bufs=1)
            q_sb = qkpool.tile([64, S], BF16, tag="q_sb", name="q_sb")
            k_sb = qkpool.tile([64, S], BF16, tag="k_sb", name="k_sb")
            ringA = tc.alloc_tile_pool(name="ringA", bufs=2)

            # ---- proj pass over x2: Q/K, V2, E_W (1 chunk behind), gathers ----
            psE2 = {}

            def emit_ew(jd):
                gd = jd // 4
                if jd % 4 == 0:
                    psE2[gd] = psA.tile([128, 512], F32, tag="ps", name="psE2")
                for hp in range(2):
                    sl = slice(128 * jd + 64 * hp, 128 * jd + 64 * hp + 64)
                    nc.tensor.matmul(
                        psE2[gd][64 * hp: 64 * hp + 64,
                                 128 * (jd % 4) + 64 * hp: 128 * (jd % 4) + 64 * hp + 64],
                        lhsT=k_sb[:, sl], rhs=q_sb[:, sl],
                        start=True, stop=True, skip_group_check=True,
                        tile_position=(0, 64 * hp),
                    )
                if jd % 4 == 3:
                    att2_g = att2[:].rearrange("p (j hq) -> p j hq", hq=128)
                    psE2_g = psE2[gd][:].rearrange("p (jl hq) -> p jl hq", hq=128)
                    for hp in range(2):
                        nc.scalar.activation(
                            out=att2_g[64 * hp: 64 * hp + 64, 4 * gd: 4 * gd + 4,
                                       64 * hp: 64 * hp + 64],
                            in_=psE2_g[64 * hp: 64 * hp + 64, :, 64 * hp: 64 * hp + 64],
                            func=mybir.ActivationFunctionType.Exp,
                        )

            for j in range(NCH):
                if j % 4 == 0:
                    x2c = ringA.tile([128, 4 * 512], BF16, tag="x2c", bufs=2, name="x2c")
                    if j == 0:
                        nc.scalar.dma_start(x2c[:, 0:512], x2p_v[:, 0:1, :])
                        nc.scalar.dma_start(x2c[:, 512:2048], x2p_v[:, 1:4, :])
                    else:
                        nc.sync.dma_start(x2c[:], x2p_v[:, j: j + 4, :])
                xc = x2c[:, 512 * (j % 4): 512 * (j % 4 + 1)]
                if j % 4 == 0:
                    # Q/K for the whole 4-chunk group: rhs spans 4 chunks per kc
                    x2c_k = x2c[:].rearrange("p (jl kc t) -> p kc jl t", kc=KC, t=128)
                    psQ4 = psB.tile([64, 512], F32, tag="psqk", name="psQ4")
                    psK4 = psB.tile([64, 512], F32, tag="psqk", name="psK4")
                    for kc in range(KC):
                        nc.tensor.matmul(
                            psQ4[:], lhsT=qkw_t[kc][:, 0:64], rhs=x2c_k[:, kc],
                            start=(kc == 0), stop=(kc == KC - 1),
                        )
                    for kc in range(KC):
                        nc.tensor.matmul(
                            psK4[:], lhsT=qkw_t[kc][:, 64:128], rhs=x2c_k[:, kc],
                            start=(kc == 0), stop=(kc == KC - 1),
                        )
                    nc.scalar.activation(
                        out=q_sb[:, 128 * j: 128 * j + 512], in_=psQ4[:],
                        func=mybir.ActivationFunctionType.Identity, bias=qkb_t[:, 0:1],
                    )
                    nc.scalar.activation(
                        out=k_sb[:, 128 * j: 128 * j + 512], in_=psK4[:],
                        func=mybir.ActivationFunctionType.Identity, bias=qkb_t[:, 1:2],
                    )
                # V2 projection (PE keeps running; E_W of previous chunk next)
                psV = psA.tile([128, 512], F32, tag="ps", name="psV2")
                for kc in range(KC):
                    nc.tensor.matmul(
                        psV[:],
                        lhsT=xc[:, 128 * kc: 128 * (kc + 1)],
                        rhs=vwtb_t[kc][:],
                        start=(kc == 0), stop=(kc == KC - 1),
                    )
                if j >= 1:
                    emit_ew(j - 1)
                nc.vector.tensor_copy(vth2[:, 512 * j: 512 * (j + 1)], psV[:])
            emit_ew(NCH - 1)

            # ---- E_H (64 mms) ----
            k_col = k_sb[:].rearrange("p (j hp em) -> p em (j hp)", hp=2, em=64)
            q_col = q_sb[:].rearrange("p (j hp em) -> p em (j hp)", hp=2, em=64)
            att1_g = att1[:].rearrange("p (j hq) -> p j hq", hq=128)
            for g8 in range(8):
                psE1 = psA.tile([128, 512], F32, tag="ps", name="psE1")
                psE1_g = psE1[:].rearrange("p (jl hq) -> p jl hq", hq=128)
                for wl in range(8):
                    w = 8 * g8 + wl
                    wp, j2l = w % 2, (w // 2) % 4
                    em = (w % 2) * 32 + w // 2
                    nc.tensor.matmul(
                        psE1[64 * wp: 64 * wp + 64,
                             128 * j2l + 64 * wp: 128 * j2l + 64 * wp + 64],
                        lhsT=k_col[:, em, :], rhs=q_col[:, em, :],
                        start=True, stop=True, skip_group_check=True,
                        tile_position=(0, 64 * wp),
                    )
                for wp in range(2):
                    nc.scalar.activation(
                        out=att1_g[64 * wp: 64 * wp + 64, 4 * g8: 4 * g8 + 4,
                                   64 * wp: 64 * wp + 64],
                        in_=psE1_g[64 * wp: 64 * wp + 64, :, 64 * wp: 64 * wp + 64],
                        func=mybir.ActivationFunctionType.Exp,
                    )

            ringA.release()
            qkpool.release()
            ringP = tc.alloc_tile_pool(name="ringP", bufs=2)

            # ---- pass-1 V1 pipeline (prologue emitted before normalize) ----
            PD = 24
            x1c_t = {}
            vt1_t = {}

            def emit_v1(jd):
                if jd % 4 == 0:
                    x1c_t[jd // 4] = ringP.tile(
                        [128, 4 * 512], BF16, tag="x1c", bufs=2, name="x1c"
                    )
                    nc.sync.dma_start(x1c_t[jd // 4][:], x1p_v[:, jd: jd + 4, :])
                xc = x1c_t[jd // 4][:, 512 * (jd % 4): 512 * (jd % 4 + 1)]
                psV = psA.tile([128, 512], F32, tag="ps", name="psV1")
                for kc in range(KC):
                    nc.tensor.matmul(
                        psV[:], lhsT=xc[:, 128 * kc: 128 * (kc + 1)],
                        rhs=vwtb_t[kc][:],
                        start=(kc == 0), stop=(kc == KC - 1),
                    )
                vt1 = ringP.tile([128, 512], BF16, tag="vt1", bufs=PD + 2, name="vt1")
                vt1_t[jd] = vt1
                nc.scalar.activation(out=vt1[:], in_=psV[:],
                                     func=mybir.ActivationFunctionType.Copy)

            # ---- colsum Z (att1 layout) -> recip -> transpose -> broadcast ----
            att2_zv = att2[:].rearrange(
                "p (j hp wp j2) -> p j2 wp (j hp)", hp=2, wp=2, j2=32
            )
            psZT = psA.tile([128, 512], F32, tag="ps", name="psZT")
            for j2 in range(NCH):
                nc.tensor.matmul(
                    psZT[:, j2: j2 + 1], lhsT=att1[:, 128 * j2: 128 * (j2 + 1)],
                    rhs=ones_col[:], start=True, stop=False, skip_group_check=True,
                )
                for wp in range(2):
                    nc.tensor.matmul(
                        psZT[64 * wp: 64 * wp + 64, j2: j2 + 1],
                        lhsT=att2_zv[:, j2, wp, :],
                        rhs=ones_col[:], start=False, stop=(wp == 1),
                        skip_group_check=True,
                    )
            with nc.allow_low_precision(reason="softmax recip in bf16"):
                nc.vector.reciprocal(zr_sb[:], psZT[:, 0:32])
            for jd in range(4):
                emit_v1(jd)
            psT1 = psA.tile([128, 1024], BF16, tag="ps", name="psT1")
            nc.tensor.transpose(psT1[0:32, 0:128], zr_sb[:], ident[:])
            nc.vector.tensor_copy(t32a[:], psT1[0:32, 0:128])
            rrow = partT1[0:1, 0:4096]
            nc.sync.dma_start(rr_d[:], t32a[:])
            nc.sync.dma_start(rrow, rr_d.rearrange("a b -> (a b)"))
            for jd in range(4, PD):
                emit_v1(jd)

            # normalize: one big partition broadcast, then per-chunk muls
            rfull = partT2[:, 0:4096]
            nc.gpsimd.partition_broadcast(rfull, rrow)
            rf_v = rfull.rearrange(
                "p (j2 wp j hp) -> p j hp wp j2", j2=32, wp=2, hp=2
            )
            for j in range(NCH):
                eng = nc.vector if j % 2 == 0 else nc.gpsimd
                eng.tensor_mul(
                    att2[:, 128 * j: 128 * (j + 1)],
                    att2[:, 128 * j: 128 * (j + 1)],
                    rf_v[:, j],
                )
            for n in range(8):
                cols = slice(512 * n, 512 * (n + 1))
                nc.vector.tensor_mul(att1[:, cols], att1[:, cols], rfull[:, cols])

            pT2_v = partT2[:].rearrange("p (cb s) -> p cb s", cb=4)
            pT1_v = partT1[:].rearrange("p (cb s) -> p cb s", cb=4)

            # ---- pass 1 main: att_W partials (V1 pipeline PD ahead) ----
            for j in range(NCH):
                if j + PD < NCH:
                    emit_v1(j + PD)
                att2c = att2[:, 128 * j: 128 * (j + 1)]
                psO2 = psA.tile([128, 512], F32, tag="ps", name="psO2")
                for cb in range(4):
                    nc.tensor.matmul(
                        psO2[:, 128 * cb: 128 * (cb + 1)],
                        lhsT=vth2[:, 512 * j + 128 * cb: 512 * j + 128 * (cb + 1)],
                        rhs=att2c, start=True, stop=True, skip_group_check=True,
                    )
                nc.vector.tensor_copy(pT2_v[:, :, 128 * j: 128 * (j + 1)], psO2[:])
                psO1 = psA.tile([128, 512], F32, tag="ps", name="psO1")
                for cb in range(4):
                    nc.tensor.matmul(
                        psO1[:, 128 * cb: 128 * (cb + 1)],
                        lhsT=vt1_t[j][:, 128 * cb: 128 * (cb + 1)],
                        rhs=att2c, start=True, stop=True, skip_group_check=True,
                    )
                nc.scalar.activation(out=pT1_v[:, :, 128 * j: 128 * (j + 1)], in_=psO1[:],
                                     func=mybir.ActivationFunctionType.Copy)

            ringP.release()
            pool2.release()
            ringQ = tc.alloc_tile_pool(name="ringQ", bufs=2)

            # ---- pass 2: att_H + combine; DVE does y2, ACT+gpsimd do y1 ----
            pT2_c = partT2[:].rearrange(
                "p (cb m wp j2) -> p j2 cb wp m", cb=4, m=64, wp=2, j2=32
            )
            pT1_c = partT1[:].rearrange(
                "p (cb m wp j2) -> p j2 cb wp m", cb=4, m=64, wp=2, j2=32
            )
            PDQ = 4
            x2wc_t, x1wc_t, vt2w_t, vt1w_t = {}, {}, {}, {}
            ys_t = {}

            def emit_vw(jd):
                if jd % 4 == 0:
                    x2wc_t[jd // 4] = ringQ.tile([128, 4 * 512], BF16, tag="x2wc",
                                                 bufs=2, name="x2wc")
                    x1wc_t[jd // 4] = ringQ.tile([128, 4 * 512], BF16, tag="x1wc",
                                                 bufs=2, name="x1wc")
                    if jd == 0:
                        nc.sync.dma_start(x2wc_t[0][:, 0:512], x2pw_v[:, 0:1, :])
                        nc.scalar.dma_start(x1wc_t[0][:, 0:512], x1pw_v[:, 0:1, :])
                        nc.sync.dma_start(x2wc_t[0][:, 512:2048], x2pw_v[:, 1:4, :])
                        nc.scalar.dma_start(x1wc_t[0][:, 512:2048], x1pw_v[:, 1:4, :])
                    else:
                        nc.sync.dma_start(x2wc_t[jd // 4][:], x2pw_v[:, jd: jd + 4, :])
                        nc.sync.dma_start(x1wc_t[jd // 4][:], x1pw_v[:, jd: jd + 4, :])
                jl = jd % 4
                psVw2 = psA.tile([128, 512], F32, tag="ps", name="psVw2")
                for kc in range(KC):
                    nc.tensor.matmul(
                        psVw2[:],
                        lhsT=x2wc_t[jd // 4][:, 512 * jl + 128 * kc: 512 * jl + 128 * (kc + 1)],
                        rhs=vwtb_t[kc][:],
                        start=(kc == 0), stop=(kc == KC - 1),
                    )
                vt2w = ringQ.tile([128, 512], BF16, tag="vt2w", bufs=PDQ + 2, name="vt2w")
                vt2w_t[jd] = vt2w
                nc.scalar.activation(out=vt2w[:], in_=psVw2[:],
                                     func=mybir.ActivationFunctionType.Copy)
                psVw1 = psA.tile([128, 512], F32, tag="ps", name="psVw1")
                for kc in range(KC):
                    nc.tensor.matmul(
                        psVw1[:],
                        lhsT=x1wc_t[jd // 4][:, 512 * jl + 128 * kc: 512 * jl + 128 * (kc + 1)],
                        rhs=vwtb_t[kc][:],
                        start=(kc == 0), stop=(kc == KC - 1),
                    )
                vt1w = ringQ.tile([128, 512], BF16, tag="vt1w", bufs=PDQ + 2, name="vt1w")
                vt1w_t[jd] = vt1w
                nc.vector.tensor_copy(vt1w[:], psVw1[:])

            for jd in range(PDQ):
                emit_vw(jd)
            for j2 in range(NCH):
                gi, jj = j2 // 4, j2 % 4
                if j2 + PDQ < NCH:
                    emit_vw(j2 + PDQ)
                if jj == 0:
                    ys2 = ringQ.tile([128, 4 * 512], BF16, tag="ys2", bufs=2, name="ys2")
                    ys1 = ringQ.tile([128, 4 * 512], BF16, tag="ys1", bufs=2, name="ys1")
                att1c = att1[:, 128 * j2: 128 * (j2 + 1)]
                psF2 = psA.tile([128, 512], F32, tag="ps", name="psF2")
                for cb in range(4):
                    nc.tensor.matmul(
                        psF2[:, 128 * cb: 128 * (cb + 1)],
                        lhsT=vt2w_t[j2][:, 128 * cb: 128 * (cb + 1)],
                        rhs=att1c, start=True, stop=True, skip_group_check=True,
                    )
                psF1 = psA.tile([128, 512], F32, tag="ps", name="psF1")
                for cb in range(4):
                    nc.tensor.matmul(
                        psF1[:, 128 * cb: 128 * (cb + 1)],
                        lhsT=vt1w_t[j2][:, 128 * cb: 128 * (cb + 1)],
                        rhs=att1c, start=True, stop=True, skip_group_check=True,
                    )
                ys2_v = ys2[:].rearrange(
                    "p (cb jl wp m) -> p jl cb wp m", cb=4, jl=4, wp=2, m=64
                )
                ys1_v = ys1[:].rearrange(
                    "p (cb jl wp m) -> p jl cb wp m", cb=4, jl=4, wp=2, m=64
                )
                psF2_v = psF2[:].rearrange("p (cb wp m) -> p cb wp m", cb=4, wp=2, m=64)
                psF1_v = psF1[:].rearrange("p (cb wp m) -> p cb wp m", cb=4, wp=2, m=64)
                if j2 >= NCH - 4:
                    f2tmp = ringQ.tile([128, 512], BF16, tag="f1tmp", bufs=3, name="f2tmp")
                    nc.scalar.activation(out=f2tmp[:], in_=psF2[:],
                                         func=mybir.ActivationFunctionType.Copy)
                    f2_v = f2tmp[:].rearrange("p (cb wp m) -> p cb wp m", cb=4, wp=2, m=64)
                    nc.gpsimd.tensor_add(ys2_v[:, jj], f2_v, pT2_c[:, j2])
                    nc.vector.tensor_add(ys1_v[:, jj], psF1_v, pT1_c[:, j2])
                else:
                    nc.vector.tensor_add(ys2_v[:, jj], psF2_v, pT2_c[:, j2])
                    f1tmp = ringQ.tile([128, 512], BF16, tag="f1tmp", bufs=3, name="f1tmp")
                    nc.scalar.activation(out=f1tmp[:], in_=psF1[:],
                                         func=mybir.ActivationFunctionType.Copy)
                    f1_v = f1tmp[:].rearrange("p (cb wp m) -> p cb wp m", cb=4, wp=2, m=64)
                    nc.gpsimd.tensor_add(ys1_v[:, jj], f1_v, pT1_c[:, j2])
                if j2 < 24 and jj == 3:
                    nc.gpsimd.dma_start(y2t_v[gi], ys2[:])
                    nc.sync.dma_start(y1t_v[gi], ys1[:])
                elif j2 >= 24 and j2 % 2 == 1:
                    half = (jj - 1) // 2
                    y2h = y2t_v[gi].rearrange("p cb (hf sg) -> p hf cb sg", hf=2)
                    y1h = y1t_v[gi].rearrange("p cb (hf sg) -> p hf cb sg", hf=2)
                    ys2h = ys2[:].rearrange("p (cb hf sg) -> p hf cb sg", cb=4, hf=2)
                    ys1h = ys1[:].rearrange("p (cb hf sg) -> p hf cb sg", cb=4, hf=2)
                    nc.gpsimd.dma_start(y2h[:, half], ys2h[:, half])
                    nc.sync.dma_start(y1h[:, half], ys1h[:, half])

            ringQ.release()

    nc.compile()
    return nc


def make_in_maps(x2, x1, q_w, q_b, k_w, k_b, v_w, v_b, gamma):
    x2 = np.asarray(x2, dtype=np.float32)
    x1 = np.asarray(x1, dtype=np.float32)
    g = float(np.asarray(gamma).reshape(-1)[0])
    bf16 = ml_dtypes.bfloat16
    qkw = (
        np.concatenate([np.asarray(q_w).T, np.asarray(k_w).T], axis=1)
        .reshape(KC, 128, 128).astype(bf16)
    )
    qkb2 = np.stack([np.asarray(q_b), np.asarray(k_b)], axis=1).astype(bf16)
    vwtb = (g * np.asarray(v_w)).T.reshape(KC, 128, C).astype(bf16)

    def pack_p(xfl):
        t = xfl.reshape(KC, 128, NCH, 2, 32, 2)  # kc ch j hp u wpar
        return np.ascontiguousarray(
            t.transpose(2, 1, 0, 3, 5, 4).reshape(NCH, 128, KC * 128).astype(bf16)
        )

    def pack_pw(xfl):
        t = xfl.reshape(KC, 128, 64, 32, 2)  # kc ch h j2 wq
        return np.ascontiguousarray(
            t.transpose(3, 1, 0, 4, 2).reshape(NCH, 128, KC * 128).astype(bf16)
        )

    in_maps = []
    for b in range(B):
        x2fl = x2[b].reshape(C, S)
        x1fl = x1[b].reshape(C, S)
        in_maps.append(
            {
                "x2p": pack_p(x2fl),
                "x1p": pack_p(x1fl),
                "x2pw": pack_pw(x2fl),
                "x1pw": pack_pw(x1fl),
                "qkw": qkw,
                "qkb2": qkb2,
                "vwtb": vwtb,
            }
        )
    return in_maps, g


def assemble_outputs(res, x2, x1, v_b, g):
    y2 = np.empty((B, C, H, W), np.float32)
    y1 = np.empty((B, C, H, W), np.float32)
    gvb = (g * np.asarray(v_b, dtype=np.float32))[None, :, None, None]
    for b in range(B):
        y2[b] = unpermute(np.asarray(res[b]["y2t"]))
        y1[b] = unpermute(np.asarray(res[b]["y1t"]))
    y2 += gvb
    y2 += np.asarray(x2, dtype=np.float32)
    y1 += gvb
    y1 += np.asarray(x1, dtype=np.float32)
    return y2, y1


def unpermute(yt):
    # yt [C, s''=j2*128+wp*64+h] -> y[c, h, w=2*j2+wp]
    return np.ascontiguousarray(
        yt.astype(np.float32).reshape(C, 32, 2, 64).transpose(0, 3, 1, 2).reshape(C, H, W)
    )


def kernel(x2, x1, q_w, q_b, k_w, k_b, v_w, v_b, gamma):
    in_maps, g = make_in_maps(x2, x1, q_w, q_b, k_w, k_b, v_w, v_b, gamma)
    if "nc" not in _CACHED:
        _CACHED["nc"] = build_nc()
    nc = _CACHED["nc"]
    res = run_bass_kernel_spmd(nc, in_maps, list(range(B))).results
    return assemble_outputs(res, x2, x1, v_b, g)
